# revision 1
# baseline (speedup 1.0000x reference)
"""ODE-RNN Trainium2 Bass kernel — v3 (fp16 matmuls, skewed group pipeline,
PE-distributed GRU algebra).

Data-parallel over batch across 8 NeuronCores (4 batches x 34 slots = 136
rows/core).  State kept transposed [DRNN=128 partitions, rows on free dim].
Matmuls run in fp16 (1 cyc/row vs 4 for fp32 on the PE) with per-site
precision modes:
  f16    -- both operands fp16 (one matmul)
  f16x2w -- W split hi/lo fp16, moving single fp16 (2 matmuls)
  f16x3  -- W and moving both split hi/lo fp16 (3 matmuls, ~fp32)
  f32    -- full fp32 (4 cyc/row)
ODE Euler steps telescoped through PSUM accumulation (z1 += DT*w2w0^T a2).
GRU1 z-gate weights negated host-side so sigmoid directly yields 1-z.
Vector-op chain hops are traded for PSUM-accumulated matmuls:
  wih1^T h2   = wih1^T (zz0*h1) [early] + wih1^T (u0*n0) [post-n0]
  w0^T h_next = w0^T q [pre-n1] + w0^T p1 [post-n1],  q = h1*(1-g), p1 = g*n1
so the dependency chain never waits on h2 or h_next materialization.

Scheduling: engine SEQs are strict FIFO; dependency waits are blocking
EventSemaphores.  The G row-groups are emitted PHASE-INTERLEAVED with a
stagger of PH/G phases so that while group 0 waits on its activation,
group 1's matmuls (already dependency-satisfied) sit next in the stream.

PSUM (8 banks of 2KB): per group-slot 4 banks; long-open accumulation
groups (z1 blocks, hd) each own a bank, quick open/close pairs share
banks strictly back-to-back (HW-verified invariant from v1).
"""

import os
import numpy as np
import ml_dtypes

B, S, P, J = 32, 128, 2, 17
DIN, DOUT, DRNN, DHID = 2, 3, 128, 256
N = P * J            # 34
DT = 0.1
K = 4                # Euler steps
NCORES = 8
BL = B // NCORES     # 4 batches per core
R = BL * N           # 136 rows per core

G = int(os.environ.get("ODERNN_G", "2"))
NSTEPS = int(os.environ.get("ODERNN_STEPS", S))
SR = NSTEPS * R

M_Z2 = os.environ.get("ODERNN_M_Z2", "f16x2w")      # w1^T a1  (undamped)
M_W20 = os.environ.get("ODERNN_M_W20", "f16x2w")  # W20/w2^T a2 (DT-damped)
M_Z1 = os.environ.get("ODERNN_M_Z1", "f32")      # w0^T h   (z1 init)
M_G0H = os.environ.get("ODERNN_M_G0H", "f32")    # whh0^T h1
M_G1H = os.environ.get("ODERNN_M_G1H", "f32")    # whh1^T h1
M_G1I = os.environ.get("ODERNN_M_G1I", "f32")    # wih1^T h2

F16 = np.float16

_prog_cache = {}


def _split16(a):
    hi = a.astype(F16)
    lo = (a.astype(np.float32) - hi.astype(np.float32)).astype(F16)
    return hi, lo


def _build_program(flags):
    import concourse.tile as tile
    import concourse.mybir as mybir
    from concourse import bacc

    (fb01, fb2, frz0, fg0h, fg0i, frz1, fg1h, fg1i, fbout) = flags

    dt = mybir.dt
    f32 = dt.float32
    f16 = dt.float16
    Alu = mybir.AluOpType
    Act = mybir.ActivationFunctionType

    nc = bacc.Bacc("TRN2", target_bir_lowering=False)

    RG = R // G
    assert R % G == 0 and RG <= 120

    def wdt(mode):
        return f32 if mode == "f32" else f16

    # ---- DRAM I/O ----
    d_xm6 = nc.dram_tensor("xm6", [6, SR], f16, kind="ExternalInput")
    d_mm = nc.dram_tensor("mm16", [DRNN, SR], f16, kind="ExternalInput")
    d_h0f = nc.dram_tensor("h0f", [DRNN, R], f32, kind="ExternalInput")
    d_h0h = nc.dram_tensor("h0h", [DRNN, R], f16, kind="ExternalInput")
    d_h0l = nc.dram_tensor("h0l", [DRNN, R], f16, kind="ExternalInput")

    dram_w = {}

    def wdecl(name, shape, mode):
        if mode in ("f16x2w", "f16x3"):
            dram_w[name + "h"] = nc.dram_tensor(name + "h", shape, f16,
                                                kind="ExternalInput")
            dram_w[name + "l"] = nc.dram_tensor(name + "l", shape, f16,
                                                kind="ExternalInput")
        else:
            dram_w[name] = nc.dram_tensor(name, shape, wdt(mode),
                                          kind="ExternalInput")

    wdecl("w0", [DRNN, DHID], M_Z1)
    wdecl("w1a", [128, DHID], M_Z2)
    wdecl("w1b", [128, DHID], M_Z2)
    wdecl("w2a", [128, DRNN], M_W20)
    wdecl("w2b", [128, DRNN], M_W20)
    wdecl("W20a", [128, DHID], M_W20)
    wdecl("W20b", [128, DHID], M_W20)
    wdecl("whh0", [DRNN, 3 * DRNN], M_G0H)
    wdecl("whh1", [DRNN, 3 * DRNN], M_G1H)   # z-gate cols negated
    wdecl("wih1", [DRNN, 3 * DRNN], M_G1I)   # z-gate cols negated
    dram_w["wih6"] = nc.dram_tensor("wih6", [6, 3 * DRNN], f16,
                                    kind="ExternalInput")
    dram_w["wouth"] = nc.dram_tensor("wouth", [DRNN, DOUT], f16,
                                     kind="ExternalInput")
    d_bias = nc.dram_tensor("biaspk", [DRNN, 24], f32, kind="ExternalInput")
    d_y = nc.dram_tensor("y", [DOUT, SR], f32, kind="ExternalOutput")

    with tile.TileContext(nc) as tc:
        wp = tc.alloc_tile_pool(name="wconst", bufs=1)
        st = tc.alloc_tile_pool(name="state", bufs=4)
        wk = tc.alloc_tile_pool(name="work", bufs=6)
        pp = tc.alloc_tile_pool(name="psum", bufs=1, space="PSUM")

        def load(dram, shape, dtype, name):
            t = wp.tile(shape, dtype, tag=name, name=name)
            nc.sync.dma_start(out=t[:], in_=dram[:])
            return t

        W = {}
        for nm, dten in dram_w.items():
            W[nm] = load(dten, list(dten.shape), dten.dtype, nm)
        xm6 = load(d_xm6, [6, SR], f16, "xm6")
        h0f = load(d_h0f, [DRNN, R], f32, "h0f")
        h0h = load(d_h0h, [DRNN, R], f16, "h0h")
        bias = load(d_bias, [DRNN, 24], f32, "biaspk")
        need_hl = M_Z1 == "f16x3"
        need_h1l = "f16x3" in (M_G0H, M_G1H)
        h0l = load(d_h0l, [DRNN, R], f16, "h0l") if need_hl else None
        mm16 = load(d_mm, [DRNN, SR], f16, "mm16")

        hist = wp.tile([DRNN, NSTEPS, R], f16, tag="hist", name="hist")

        MM = nc.tensor.matmul
        ACT = nc.scalar.activation
        V = nc.vector

        # ---- PSUM: NSLOT slots x {tz1 (2 banks), tAB (2 banks)} ----
        # Long-open groups (z1 blocks, hd) own their banks; quick pairs
        # share strictly back-to-back.  prz1 blocks live in SEPARATE banks
        # (same region index of tAB's two banks) so both can be opened
        # early by the whh1 matmuls while staying one-open-per-bank, and
        # the merged rz1 ACT still sees one strided AP.
        NSLOT = 2
        slots = []
        for si in range(NSLOT):
            tz1 = pp.tile([128, 2, 512], f32, tag=f"tz1_{si}",
                          name=f"tz1_{si}")
            tAB = pp.tile([128, 2, 4, 128], f32, tag=f"tAB_{si}",
                          name=f"tAB_{si}")
            slots.append({
                "z1": tz1[:, :, 0:RG],
                "z1b": [tz1[:, 0, 0:RG], tz1[:, 1, 0:RG]],
                "prz0": tz1[:, :, 128:128 + RG],
                "prz0b": [tz1[:, 0, 128:128 + RG], tz1[:, 1, 128:128 + RG]],
                "pg0_ig": tz1[:, 0, 256:256 + RG],
                "pg0_hg": tz1[:, 1, 256:256 + RG],
                "pg1_ig": tz1[:, 0, 384:384 + RG],
                "z2": tAB[:, 0, 0:2, 0:RG],
                "z2b": [tAB[:, 0, 0, 0:RG], tAB[:, 0, 1, 0:RG]],
                "prz1": tAB[:, :, 2, 0:RG],
                "prz1b": [tAB[:, 0, 2, 0:RG], tAB[:, 1, 2, 0:RG]],
                "hd": tAB[:, 1, 0, 0:RG],
                "pg1_hg": tAB[:, 1, 1, 0:RG],
                "py": tAB[0:DOUT, 1, 3, 0:RG],
            })

        def site_mm(out, wname, blk, mov_f32, mov_h, mov_l, mode,
                    start, stop):
            lo, hi = blk
            if mode == "f32":
                MM(out, W[wname][:, lo:hi], mov_f32, start=start, stop=stop)
            elif mode == "f16":
                MM(out, W[wname][:, lo:hi], mov_h, start=start, stop=stop)
            elif mode == "f16x2w":
                MM(out, W[wname + "h"][:, lo:hi], mov_h, start=start,
                   stop=False)
                MM(out, W[wname + "l"][:, lo:hi], mov_h, start=False,
                   stop=stop)
            else:  # f16x3
                MM(out, W[wname + "h"][:, lo:hi], mov_h, start=start,
                   stop=False)
                MM(out, W[wname + "h"][:, lo:hi], mov_l, start=False,
                   stop=False)
                MM(out, W[wname + "l"][:, lo:hi], mov_h, start=False,
                   stop=stop)

        a1dt = f32 if M_Z2 == "f32" else f16
        a2dt = f32 if M_W20 == "f32" else f16

        def group_body(gi):
            """Generator emitting one group's full sequence; yields at
            phase boundaries for cross-group interleaving."""
            hpf = h0f[:, gi * RG:(gi + 1) * RG]
            hph = h0h[:, gi * RG:(gi + 1) * RG]
            hpl = h0l[:, gi * RG:(gi + 1) * RG] if need_hl else None

            for s in range(NSTEPS):
                t = slots[(s * G + gi) % NSLOT]
                c0 = s * R + gi * RG
                c1 = c0 + RG
                xsl = xm6[:, c0:c1]
                msl = mm16[:, c0:c1]

                # ph0: open z1 telescopes.  For s>0 with M_Z1=f32 this
                # already happened during the previous step's tail:
                # z1 = w0^T h_next = w0^T q + w0^T p1 distributed through
                # the PE, so the chain never waits on an h_next vector op.
                if s == 0 or M_Z1 != "f32":
                    site_mm(t["z1b"][0], "w0", (0, 128), hpf, hph, hpl,
                            M_Z1, True, False)
                    site_mm(t["z1b"][1], "w0", (128, 256), hpf, hph, hpl,
                            M_Z1, True, False)
                yield

                for k in range(K):
                    last = k == K - 1
                    a1 = wk.tile([128, 2, RG], a1dt, tag=f"a1_{gi}")
                    if not fb01:
                        ACT(a1[:], t["z1"], Act.Tanh)
                    else:
                        ACT(a1[:, 0, :], t["z1b"][0], Act.Tanh,
                            bias=bias[:, 16 + 2 * k:17 + 2 * k])
                        ACT(a1[:, 1, :], t["z1b"][1], Act.Tanh,
                            bias=bias[:, 17 + 2 * k:18 + 2 * k])
                    yield
                    a1h = [a1[:, 0, :], a1[:, 1, :]]
                    site_mm(t["z2b"][0], "w1a", (0, 128), a1h[0], a1h[0],
                            None, M_Z2, True, False)
                    site_mm(t["z2b"][0], "w1b", (0, 128), a1h[1], a1h[1],
                            None, M_Z2, False, True)
                    site_mm(t["z2b"][1], "w1a", (128, 256), a1h[0], a1h[0],
                            None, M_Z2, True, False)
                    site_mm(t["z2b"][1], "w1b", (128, 256), a1h[1], a1h[1],
                            None, M_Z2, False, True)
                    yield
                    a2 = wk.tile([128, 2, RG], a2dt, tag=f"a2_{gi}")
                    if not fb01:
                        ACT(a2[:], t["z2"], Act.Tanh)
                    else:
                        ACT(a2[:, 0, :], t["z2b"][0], Act.Tanh,
                            bias=bias[:, 2:3])
                        ACT(a2[:, 1, :], t["z2b"][1], Act.Tanh,
                            bias=bias[:, 3:4])
                    yield
                    a2h = [a2[:, 0, :], a2[:, 1, :]]
                    if not last:
                        fin = k == K - 2
                        site_mm(t["z1b"][0], "W20a", (0, 128), a2h[0],
                                a2h[0], None, M_W20, False, False)
                        site_mm(t["z1b"][0], "W20b", (0, 128), a2h[1],
                                a2h[1], None, M_W20, False, fin)
                        site_mm(t["z1b"][1], "W20a", (128, 256), a2h[0],
                                a2h[0], None, M_W20, False, False)
                        site_mm(t["z1b"][1], "W20b", (128, 256), a2h[1],
                                a2h[1], None, M_W20, False, fin)
                    site_mm(t["hd"], "w2a", (0, 128), a2h[0], a2h[0], None,
                            M_W20, k == 0, False)
                    site_mm(t["hd"], "w2b", (0, 128), a2h[1], a2h[1], None,
                            M_W20, False, last)
                    yield

                # h1 = h_prev + (hd + DT*b2)
                h1f = st.tile([128, RG], f32, tag=f"h1f_{gi}")
                V.scalar_tensor_tensor(h1f[:], t["hd"], bias[:, 4:5], hpf,
                                       op0=Alu.add, op1=Alu.add)
                h1h = hist[:, s, gi * RG:(gi + 1) * RG]
                V.tensor_copy(h1h, h1f[:])
                h1l = None
                if need_h1l:
                    h1lt = wk.tile([128, RG], f16, tag=f"h1l_{gi}")
                    V.tensor_sub(h1lt[:], h1f[:], h1h)
                    h1l = h1lt[:]
                yield

                # GRU0 matmuls + GRU1 g-gate hidden side + wout projection
                MM(t["pg0_ig"], W["wih6"][:, 256:384], xsl, start=True,
                   stop=True)
                # dep-free x-openers first (the two prz0 blocks live in
                # different banks, so both groups may be open at once);
                # the h1f-gated whh0 closers run back-to-back after
                MM(t["prz0b"][0], W["wih6"][:, 0:128], xsl, start=True,
                   stop=False)
                MM(t["prz0b"][1], W["wih6"][:, 128:256], xsl, start=True,
                   stop=False)
                site_mm(t["prz0b"][0], "whh0", (0, 128), h1f[:], h1h, h1l,
                        M_G0H, False, True)
                site_mm(t["prz0b"][1], "whh0", (128, 256), h1f[:], h1h, h1l,
                        M_G0H, False, True)
                site_mm(t["pg0_hg"], "whh0", (256, 384), h1f[:], h1h, h1l,
                        M_G0H, True, True)
                site_mm(t["pg1_hg"], "whh1", (256, 384), h1f[:], h1h, h1l,
                        M_G1H, True, True)
                MM(t["py"], W["wouth"][:], h1h, start=True, stop=True)
                # open GRU1 r/z groups on the h1-dependent halves now; the
                # wih1 input-side terms arrive as v0/p0 products later
                # (h2 = u0*n0 + zz0*h1 distributed through the PE)
                site_mm(t["prz1b"][0], "whh1", (0, 128), h1f[:], h1h, h1l,
                        M_G1H, True, False)
                site_mm(t["prz1b"][1], "whh1", (128, 256), h1f[:], h1h, h1l,
                        M_G1H, True, False)
                yield

                rz0 = wk.tile([128, 2, RG], f32, tag=f"rz0_{gi}")
                if not frz0:
                    ACT(rz0[:], t["prz0"], Act.Sigmoid)
                else:
                    ACT(rz0[:, 0, :], t["prz0b"][0], Act.Sigmoid,
                        bias=bias[:, 5:6])
                    ACT(rz0[:, 1, :], t["prz0b"][1], Act.Sigmoid,
                        bias=bias[:, 6:7])
                r0, zz0 = rz0[:, 0, :], rz0[:, 1, :]
                yield

                hg0 = t["pg0_hg"]
                if fg0h:
                    hg0t = wk.tile([128, RG], f32, tag=f"hg0t_{gi}")
                    nc.scalar.add(hg0t[:], hg0, bias[:, 8:9])
                    hg0 = hg0t[:]
                # chain: s1 -> np0 -> [n0]; u0/v precomputed for the tail
                s1 = wk.tile([128, RG], f32, tag=f"s1_{gi}")
                V.tensor_mul(s1[:], r0, hg0)
                np0 = wk.tile([128, RG], f32, tag=f"np0_{gi}")
                V.tensor_add(np0[:], s1[:], t["pg0_ig"])
                u0 = wk.tile([128, RG], f32, tag=f"u0_{gi}")
                V.tensor_scalar(u0[:], zz0, -1.0, 1.0, op0=Alu.mult,
                                op1=Alu.add)
                g1dt = f32 if M_G1I == "f32" else f16
                v0 = wk.tile([128, RG], g1dt, tag=f"v0_{gi}")
                V.tensor_mul(v0[:], zz0, h1f[:])
                yield

                n0 = wk.tile([128, RG], f32, tag=f"n0_{gi}")
                if not fg0i:
                    ACT(n0[:], np0[:], Act.Tanh)
                else:
                    ACT(n0[:], np0[:], Act.Tanh, bias=bias[:, 7:8])
                # v0-half of the GRU1 input-side products
                MM(t["prz1b"][0], W["wih1"][:, 0:128], v0[:], start=False,
                   stop=False)
                MM(t["prz1b"][1], W["wih1"][:, 128:256], v0[:], start=False,
                   stop=False)
                MM(t["pg1_ig"], W["wih1"][:, 256:384], v0[:], start=True,
                   stop=False)
                yield

                # only p0 = u0*n0 sits on the chain after n0
                p0 = wk.tile([128, RG], g1dt, tag=f"p0_{gi}")
                V.tensor_mul(p0[:], u0[:], n0[:])
                # y slice out (off critical path, backfills this phase)
                ysl = wk.tile([DOUT, RG], f32, tag=f"ysl_{gi}")
                if not fbout:
                    V.tensor_copy(ysl[:], t["py"])
                else:
                    ACT(ysl[:], t["py"], Act.Identity,
                        bias=bias[0:DOUT, 15:16])
                nc.sync.dma_start(out=d_y[:, c0:c1], in_=ysl[:])
                yield

                # close GRU1 groups with the p0-half products
                MM(t["prz1b"][0], W["wih1"][:, 0:128], p0[:], start=False,
                   stop=True)
                MM(t["prz1b"][1], W["wih1"][:, 128:256], p0[:], start=False,
                   stop=True)
                MM(t["pg1_ig"], W["wih1"][:, 256:384], p0[:], start=False,
                   stop=True)
                yield

                rz1 = wk.tile([128, 2, RG], f32, tag=f"rz1_{gi}")
                if not frz1:
                    ACT(rz1[:], t["prz1"], Act.Sigmoid)
                else:
                    ACT(rz1[:, 0, :], t["prz1b"][0], Act.Sigmoid,
                        bias=bias[:, 9:10])
                    ACT(rz1[:, 1, :], t["prz1b"][1], Act.Sigmoid,
                        bias=bias[:, 10:11])
                r1, u1 = rz1[:, 0, :], rz1[:, 1, :]
                yield

                hg1 = t["pg1_hg"]
                if fg1h:
                    hg1t = wk.tile([128, RG], f32, tag=f"hg1t_{gi}")
                    nc.scalar.add(hg1t[:], hg1, bias[:, 12:13])
                    hg1 = hg1t[:]
                # chain: s2 -> np1 -> [n1]; gm/gh/q precomputed for the tail
                s2 = wk.tile([128, RG], f32, tag=f"s2_{gi}")
                V.tensor_mul(s2[:], r1, hg1)
                np1 = wk.tile([128, RG], f32, tag=f"np1_{gi}")
                V.tensor_add(np1[:], s2[:], t["pg1_ig"])
                gm = wk.tile([128, RG], f32, tag=f"gm_{gi}")
                V.tensor_mul(gm[:], u1, msl)
                gh = wk.tile([128, RG], f32, tag=f"gh_{gi}")
                V.tensor_mul(gh[:], gm[:], h1f[:])
                q = st.tile([128, RG], f32, tag=f"q_{gi}")
                V.tensor_sub(q[:], h1f[:], gh[:])
                if M_Z1 == "f32" and s + 1 < NSTEPS:
                    # open next step's z1 telescope early on the q-half
                    tn = slots[((s + 1) * G + gi) % NSLOT]
                    MM(tn["z1b"][0], W["w0"][:, 0:128], q[:], start=True,
                       stop=False)
                    MM(tn["z1b"][1], W["w0"][:, 128:256], q[:], start=True,
                       stop=False)
                yield

                n1 = wk.tile([128, RG], f32, tag=f"n1_{gi}")
                if not fg1i:
                    ACT(n1[:], np1[:], Act.Tanh)
                else:
                    ACT(n1[:], np1[:], Act.Tanh, bias=bias[:, 11:12])
                yield

                # h_next = h1*(1-g) + g*n1 = q + g*n1, g = mm*u1
                p1 = st.tile([128, RG], f32, tag=f"p1_{gi}")
                V.tensor_mul(p1[:], gm[:], n1[:])
                if M_Z1 == "f32" and s + 1 < NSTEPS:
                    # p1-half of next step's z1 telescope (chain never
                    # touches hn itself)
                    tn = slots[((s + 1) * G + gi) % NSLOT]
                    MM(tn["z1b"][0], W["w0"][:, 0:128], p1[:], start=False,
                       stop=False)
                    MM(tn["z1b"][1], W["w0"][:, 128:256], p1[:],
                       start=False, stop=False)
                hn = st.tile([128, RG], f32, tag=f"hn_{gi}")
                V.tensor_add(hn[:], q[:], p1[:])
                hpf = hn[:]
                if M_Z1 in ("f16", "f16x2w", "f16x3"):
                    hnh = st.tile([128, RG], f16, tag=f"hnh_{gi}")
                    V.tensor_copy(hnh[:], hn[:])
                    hph = hnh[:]
                    if need_hl:
                        hnl = st.tile([128, RG], f16, tag=f"hnl_{gi}")
                        V.tensor_sub(hnl[:], hn[:], hnh[:])
                        hpl = hnl[:]
                yield

        # staggered round-robin phase interleave across groups
        PH = 11 + 4 * K         # yields per step
        STAG = int(os.environ.get("ODERNN_STAG", "20"))
        gens = [group_body(gi) for gi in range(G)]
        started = [False] * G
        done = [False] * G
        tick = 0
        while not all(done):
            for gi in range(G):
                if tick >= gi * STAG and not done[gi]:
                    started[gi] = True
                    try:
                        next(gens[gi])
                    except StopIteration:
                        done[gi] = True
            tick += 1

        pp.release()
        wk.release()
        st.release()
        wp.release()

    nc.compile()
    return nc


def _prep(inputs):
    x2d = np.asarray(inputs["x2d"], np.float32)
    mask = np.asarray(inputs["mask"])
    g = lambda n: np.asarray(inputs[n], np.float32)
    w0, b0 = g("ode_w0"), g("ode_b0")
    w1, b1 = g("ode_w1"), g("ode_b1")
    w2, b2 = g("ode_w2"), g("ode_b2")
    wih0, whh0 = g("wih0"), g("whh0")
    bih0, bhh0 = g("bih0"), g("bhh0")
    wih1, whh1 = g("wih1"), g("whh1")
    bih1, bhh1 = g("bih1"), g("bhh1")
    wout, bout = g("wout"), g("bout")
    h0 = g("h0")

    mf = mask.astype(np.float32)
    xs = (x2d * mf).reshape(B, S, N, DIN)[:, :NSTEPS]
    ms = mf.reshape(B, S, N)[:, :NSTEPS]

    W20 = (DT * (w2.astype(np.float64) @ w0.astype(np.float64))).astype(np.float32)
    h0T = np.repeat(h0.reshape(DRNN, 1), R, axis=1).astype(np.float32)
    h0h, h0l = _split16(h0T)

    whh1n = whh1.copy(); whh1n[:, 128:256] *= -1.0
    wih1n = wih1.copy(); wih1n[:, 128:256] *= -1.0

    bp = np.zeros((DRNN, 24), np.float32)
    bp[:, 0], bp[:, 1] = b0[0:128], b0[128:256]
    bp[:, 2], bp[:, 3] = b1[0:128], b1[128:256]
    bp[:, 4] = DT * b2
    brz0 = bih0 + bhh0
    bp[:, 5], bp[:, 6] = brz0[0:128], brz0[128:256]
    bp[:, 7] = bih0[256:384]
    bp[:, 8] = bhh0[256:384]
    brz1 = bih1 + bhh1
    bp[:, 9] = brz1[0:128]
    bp[:, 10] = -brz1[128:256]          # z-gate negated
    bp[:, 11] = bih1[256:384]
    bp[:, 12] = bhh1[256:384]
    bp[0:DOUT, 15] = bout
    zb = DT * (b2 @ w0)
    for k in range(K):
        bp[:, 16 + 2 * k + 0] = b0[0:128] + k * zb[0:128]
        bp[:, 16 + 2 * k + 1] = b0[128:256] + k * zb[128:256]

    flags = (
        bool(np.any(b0) or np.any(b1) or np.any(b2)),
        bool(np.any(b2)),
        bool(np.any(brz0[0:256])),
        bool(np.any(bhh0[256:384])),
        bool(np.any(bih0[256:384])),
        bool(np.any(brz1[0:256])),
        bool(np.any(bhh1[256:384])),
        bool(np.any(bih1[256:384])),
        bool(np.any(bout)),
    )

    C = np.ascontiguousarray

    def wpack(name, arr, mode):
        out = {}
        if mode in ("f16x2w", "f16x3"):
            hi, lo = _split16(arr)
            out[name + "h"] = C(hi)
            out[name + "l"] = C(lo)
        elif mode == "f16":
            out[name] = C(arr.astype(F16))
        else:
            out[name] = C(arr.astype(np.float32))
        return out

    shared = {}
    shared.update(wpack("w0", w0, M_Z1))
    shared.update(wpack("w1a", w1[0:128], M_Z2))
    shared.update(wpack("w1b", w1[128:256], M_Z2))
    shared.update(wpack("w2a", DT * w2[0:128], M_W20))
    shared.update(wpack("w2b", DT * w2[128:256], M_W20))
    shared.update(wpack("W20a", W20[0:128], M_W20))
    shared.update(wpack("W20b", W20[128:256], M_W20))
    shared.update(wpack("whh0", whh0, M_G0H))
    shared.update(wpack("whh1", whh1n, M_G1H))
    shared.update(wpack("wih1", wih1n, M_G1I))
    wih0h, wih0l = _split16(wih0)
    shared["wih6"] = C(np.concatenate([wih0h, wih0h, wih0l], axis=0))
    shared["wouth"] = C(wout.astype(F16))
    shared["biaspk"] = bp
    shared["h0f"] = h0T
    shared["h0h"] = C(h0h)
    shared["h0l"] = C(h0l)

    in_maps = []
    for c in range(NCORES):
        xc = xs[c * BL:(c + 1) * BL]
        xmT = xc.transpose(3, 1, 0, 2).reshape(DIN, SR)
        xh, xl = _split16(xmT)
        xm6 = np.concatenate([xh, xl, xh], axis=0)
        mc = ms[c * BL:(c + 1) * BL]
        mrow = mc.transpose(1, 0, 2).reshape(1, SR)
        mbc = np.broadcast_to(mrow, (DRNN, SR)).astype(F16)
        m = dict(shared)
        m["xm6"] = C(xm6)
        m["mm16"] = C(mbc)
        in_maps.append(m)
    return in_maps, flags


def kernel(**inputs):
    in_maps, flags = _prep(inputs)
    if flags not in _prog_cache:
        _prog_cache[flags] = _build_program(flags)
    nc = _prog_cache[flags]

    from concourse.bass_utils import run_bass_kernel_spmd
    res = run_bass_kernel_spmd(nc, in_maps, core_ids=list(range(NCORES)))
    global _last_results
    _last_results = res.results

    ys = np.zeros((B, NSTEPS, P, J, DOUT), np.float32)
    for c in range(NCORES):
        y = res.results[c]["y"]                      # (DOUT, SR)
        y = y.reshape(DOUT, NSTEPS, BL, N).transpose(2, 1, 3, 0)
        ys[c * BL:(c + 1) * BL] = y.reshape(BL, NSTEPS, P, J, DOUT)
    return ys



# revision 38
# speedup vs baseline: 1.0003x; 1.0003x over previous
"""ODE-RNN Trainium2 Bass kernel — v3 (fp16 matmuls, skewed group pipeline,
PE-distributed GRU algebra).

Data-parallel over batch across 8 NeuronCores (4 batches x 34 slots = 136
rows/core).  State kept transposed [DRNN=128 partitions, rows on free dim].
Matmuls run in fp16 (1 cyc/row vs 4 for fp32 on the PE) with per-site
precision modes:
  f16    -- both operands fp16 (one matmul)
  f16x2w -- W split hi/lo fp16, moving single fp16 (2 matmuls)
  f16x3  -- W and moving both split hi/lo fp16 (3 matmuls, ~fp32)
  f32    -- full fp32 (4 cyc/row)
ODE Euler steps telescoped through PSUM accumulation (z1 += DT*w2w0^T a2).
GRU1 z-gate weights negated host-side so sigmoid directly yields 1-z.
Vector-op chain hops are traded for PSUM-accumulated matmuls:
  wih1^T h2   = wih1^T (zz0*h1) [early] + wih1^T (u0*n0) [post-n0]
  w0^T h_next = w0^T q [pre-n1] + w0^T p1 [post-n1],  q = h1*(1-g), p1 = g*n1
so the dependency chain never waits on h2 or h_next materialization.

Scheduling: engine SEQs are strict FIFO; dependency waits are blocking
EventSemaphores.  The G row-groups are emitted PHASE-INTERLEAVED with a
stagger of PH/G phases so that while group 0 waits on its activation,
group 1's matmuls (already dependency-satisfied) sit next in the stream.

PSUM (8 banks of 2KB): per group-slot 4 banks; long-open accumulation
groups (z1 blocks, hd) each own a bank, quick open/close pairs share
banks strictly back-to-back (HW-verified invariant from v1).
"""

import os
import numpy as np
import ml_dtypes

B, S, P, J = 32, 128, 2, 17
DIN, DOUT, DRNN, DHID = 2, 3, 128, 256
N = P * J            # 34
DT = 0.1
K = 4                # Euler steps
NCORES = 8
BL = B // NCORES     # 4 batches per core
R = BL * N           # 136 rows per core

G = int(os.environ.get("ODERNN_G", "2"))
NSTEPS = int(os.environ.get("ODERNN_STEPS", S))
SR = NSTEPS * R

M_Z2 = os.environ.get("ODERNN_M_Z2", "f16x2w")   # w1^T a1  (undamped)
M_W20 = os.environ.get("ODERNN_M_W20", "f16x2w") # W20/w2^T a2 (DT-damped)
M_Z1 = os.environ.get("ODERNN_M_Z1", "f32")      # w0^T h   (z1 telescope)
M_G0H = os.environ.get("ODERNN_M_G0H", "f32")    # whh0^T h1
M_G1H = os.environ.get("ODERNN_M_G1H", "f32")    # whh1^T h1 (off-chain)
M_G1I = os.environ.get("ODERNN_M_G1I", "f32")    # wih1^T h2

F16 = np.float16

_prog_cache = {}


def _split16(a):
    hi = a.astype(F16)
    lo = (a.astype(np.float32) - hi.astype(np.float32)).astype(F16)
    return hi, lo


_DITHER_PATTERNS = {
    3: {0: "qqq", 1: "qpq", 2: "pqp", 3: "ppp"},
    4: {0: "qqqq", 1: "qpqq", 2: "qpqp", 3: "pqpp", 4: "pppp"},
}


def _dither16(a, phases=4):
    """`phases` f16 tensors whose per-element duty-cycled average best
    approximates a, with slot patterns that also cancel linear drift of
    the moving operand across the cycle."""
    a = a.astype(np.float64)
    p = a.astype(F16)
    pf = p.astype(np.float64)
    # neighbor on the far side of a (or equal when exact)
    toward = np.where(a >= pf, np.float16(np.inf), np.float16(-np.inf))
    q = np.nextafter(p, toward.astype(F16))
    qf = q.astype(np.float64)
    ns = np.arange(phases + 1).reshape((-1,) + (1,) * a.ndim)
    means = (ns * pf + (phases - ns) * qf) / phases
    pick = np.argmin(np.abs(means - a), axis=0)    # n_p per element
    pats = _DITHER_PATTERNS[phases]
    outs = []
    for j in range(phases):
        use_p = np.zeros(a.shape, bool)
        for n, pat in pats.items():
            use_p |= (pick == n) & (pat[j] == "p")
        outs.append(np.where(use_p, p, q).astype(F16))
    return outs


def _build_program(flags):
    import concourse.tile as tile
    import concourse.mybir as mybir
    from concourse import bacc

    (fb01, fb2, frz0, fg0h, fg0i, frz1, fg1h, fg1i, fbout) = flags

    dt = mybir.dt
    f32 = dt.float32
    f16 = dt.float16
    Alu = mybir.AluOpType
    Act = mybir.ActivationFunctionType

    nc = bacc.Bacc("TRN2", target_bir_lowering=False)

    RG = R // G
    assert R % G == 0 and RG <= 120

    def wdt(mode):
        return f32 if mode == "f32" else f16

    # ---- DRAM I/O ----
    d_xm6 = nc.dram_tensor("xm6", [6, SR], f16, kind="ExternalInput")
    d_mm = nc.dram_tensor("mm16", [DRNN, SR], f16, kind="ExternalInput")
    d_h0f = nc.dram_tensor("h0f", [DRNN, R], f32, kind="ExternalInput")
    d_h0h = nc.dram_tensor("h0h", [DRNN, R], f16, kind="ExternalInput")
    d_h0l = nc.dram_tensor("h0l", [DRNN, R], f16, kind="ExternalInput")

    dram_w = {}

    def wdecl(name, shape, mode):
        if mode in ("f16x2w", "f16x3"):
            dram_w[name + "h"] = nc.dram_tensor(name + "h", shape, f16,
                                                kind="ExternalInput")
            dram_w[name + "l"] = nc.dram_tensor(name + "l", shape, f16,
                                                kind="ExternalInput")
        elif mode == "f16d":
            nv = 3 if name.startswith("W20") else 4
            for j in range(nv):
                dram_w[f"{name}d{j}"] = nc.dram_tensor(
                    f"{name}d{j}", shape, f16, kind="ExternalInput")
        else:
            dram_w[name] = nc.dram_tensor(name, shape, wdt(mode),
                                          kind="ExternalInput")

    wdecl("w0", [DRNN, DHID], M_Z1)
    wdecl("w1a", [128, DHID], M_Z2)
    wdecl("w1b", [128, DHID], M_Z2)
    wdecl("w2a", [128, DRNN], M_W20)
    wdecl("w2b", [128, DRNN], M_W20)
    wdecl("W20a", [128, DHID], M_W20)
    wdecl("W20b", [128, DHID], M_W20)
    wdecl("whh0", [DRNN, 3 * DRNN], M_G0H)
    wdecl("whh1", [DRNN, 3 * DRNN], M_G1H)   # z-gate cols negated
    wdecl("wih1", [DRNN, 3 * DRNN], M_G1I)   # z-gate cols negated
    dram_w["wih6"] = nc.dram_tensor("wih6", [6, 3 * DRNN], f16,
                                    kind="ExternalInput")
    dram_w["wouth"] = nc.dram_tensor("wouth", [DRNN, DOUT], f16,
                                     kind="ExternalInput")
    d_bias = nc.dram_tensor("biaspk", [DRNN, 24], f32, kind="ExternalInput")
    d_y = nc.dram_tensor("y", [DOUT, SR], f32, kind="ExternalOutput")

    with tile.TileContext(nc) as tc:
        wp = tc.alloc_tile_pool(name="wconst", bufs=1)
        st = tc.alloc_tile_pool(name="state", bufs=4)
        wk = tc.alloc_tile_pool(name="work", bufs=6)
        pp = tc.alloc_tile_pool(name="psum", bufs=1, space="PSUM")

        def load(dram, shape, dtype, name):
            t = wp.tile(shape, dtype, tag=name, name=name)
            nc.sync.dma_start(out=t[:], in_=dram[:])
            return t

        W = {}
        for nm, dten in dram_w.items():
            W[nm] = load(dten, list(dten.shape), dten.dtype, nm)
        xm6 = load(d_xm6, [6, SR], f16, "xm6")
        h0f = load(d_h0f, [DRNN, R], f32, "h0f")
        h0h = load(d_h0h, [DRNN, R], f16, "h0h")
        bias = load(d_bias, [DRNN, 24], f32, "biaspk")
        need_hl = M_Z1 == "f16x3"
        need_h1l = "f16x3" in (M_G0H, M_G1H)
        h0l = load(d_h0l, [DRNN, R], f16, "h0l") if need_hl else None
        mm16 = load(d_mm, [DRNN, SR], f16, "mm16")

        hist = wp.tile([DRNN, NSTEPS, R], f16, tag="hist", name="hist")

        MM = nc.tensor.matmul
        ACT = nc.scalar.activation
        V = nc.vector
        GP = nc.gpsimd if os.environ.get("ODERNN_GP", "1") == "1" else nc.vector

        # ---- PSUM: NSLOT slots x {tz1 (2 banks), tAB (2 banks)} ----
        # Long-open groups (z1 blocks, hd) own their banks; quick pairs
        # share strictly back-to-back.  prz1 blocks live in SEPARATE banks
        # (same region index of tAB's two banks) so both can be opened
        # early by the whh1 matmuls while staying one-open-per-bank, and
        # the merged rz1 ACT still sees one strided AP.
        NSLOT = 2
        slots = []
        for si in range(NSLOT):
            tz1 = pp.tile([128, 2, 512], f32, tag=f"tz1_{si}",
                          name=f"tz1_{si}")
            tAB = pp.tile([128, 2, 4, 128], f32, tag=f"tAB_{si}",
                          name=f"tAB_{si}")
            slots.append({
                "z1": tz1[:, :, 0:RG],
                "z1b": [tz1[:, 0, 0:RG], tz1[:, 1, 0:RG]],
                "prz0": tz1[:, :, 128:128 + RG],
                "prz0b": [tz1[:, 0, 128:128 + RG], tz1[:, 1, 128:128 + RG]],
                "pg0_ig": tz1[:, 0, 256:256 + RG],
                "pg0_hg": tz1[:, 1, 256:256 + RG],
                "pg1_ig": tz1[:, 0, 384:384 + RG],
                "z2": tAB[:, 0, 0:2, 0:RG],
                "z2b": [tAB[:, 0, 0, 0:RG], tAB[:, 0, 1, 0:RG]],
                "prz1": tAB[:, :, 2, 0:RG],
                "prz1b": [tAB[:, 0, 2, 0:RG], tAB[:, 1, 2, 0:RG]],
                "hd": tAB[:, 1, 0, 0:RG],
                "pg1_hg": tAB[:, 1, 1, 0:RG],
                "py": tAB[0:DOUT, 1, 3, 0:RG],
            })

        def site_mm(out, wname, blk, mov_f32, mov_h, mov_l, mode,
                    start, stop, par=0):
            lo, hi = blk
            if mode == "f32":
                MM(out, W[wname][:, lo:hi], mov_f32, start=start, stop=stop)
            elif mode == "f16":
                MM(out, W[wname][:, lo:hi], mov_h, start=start, stop=stop)
            elif mode == "f16d":
                nv = 3 if wname.startswith("W20") else 4
                MM(out, W[f"{wname}d{par % nv}"][:, lo:hi],
                   mov_h, start=start, stop=stop)
            elif mode == "f16x2w":
                MM(out, W[wname + "h"][:, lo:hi], mov_h, start=start,
                   stop=False)
                MM(out, W[wname + "l"][:, lo:hi], mov_h, start=False,
                   stop=stop)
            else:  # f16x3
                MM(out, W[wname + "h"][:, lo:hi], mov_h, start=start,
                   stop=False)
                MM(out, W[wname + "h"][:, lo:hi], mov_l, start=False,
                   stop=False)
                MM(out, W[wname + "l"][:, lo:hi], mov_h, start=False,
                   stop=stop)

        a1dt = f32 if M_Z2 == "f32" else f16
        a2dt = f32 if M_W20 == "f32" else f16

        def group_body(gi):
            """Generator emitting one group's full sequence; yields at
            phase boundaries for cross-group interleaving."""
            hpf = h0f[:, gi * RG:(gi + 1) * RG]
            hph = h0h[:, gi * RG:(gi + 1) * RG]
            hpl = h0l[:, gi * RG:(gi + 1) * RG] if need_hl else None

            for s in range(NSTEPS):
                t = slots[(s * G + gi) % NSLOT]
                c0 = s * R + gi * RG
                c1 = c0 + RG
                xsl = xm6[:, c0:c1]
                msl = mm16[:, c0:c1]

                # ph0: open z1 telescopes.  For s>0 with M_Z1=f32 this
                # already happened during the previous step's tail:
                # z1 = w0^T h_next = w0^T q + w0^T p1 distributed through
                # the PE, so the chain never waits on an h_next vector op.
                if s == 0:
                    site_mm(t["z1b"][0], "w0", (0, 128), hpf, hph, hpl,
                            M_Z1, True, False)
                    site_mm(t["z1b"][1], "w0", (128, 256), hpf, hph, hpl,
                            M_Z1, True, False)
                yield

                for k in range(K):
                    last = k == K - 1
                    a1 = wk.tile([128, 2, RG], a1dt, tag=f"a1_{gi}")
                    if not fb01:
                        ACT(a1[:], t["z1"], Act.Tanh)
                    else:
                        ACT(a1[:, 0, :], t["z1b"][0], Act.Tanh,
                            bias=bias[:, 16 + 2 * k:17 + 2 * k])
                        ACT(a1[:, 1, :], t["z1b"][1], Act.Tanh,
                            bias=bias[:, 17 + 2 * k:18 + 2 * k])
                    yield
                    a1h = [a1[:, 0, :], a1[:, 1, :]]
                    par = (s * K + k) & 3
                    site_mm(t["z2b"][0], "w1a", (0, 128), a1h[0], a1h[0],
                            None, M_Z2, True, False, par)
                    site_mm(t["z2b"][0], "w1b", (0, 128), a1h[1], a1h[1],
                            None, M_Z2, False, True, par)
                    site_mm(t["z2b"][1], "w1a", (128, 256), a1h[0], a1h[0],
                            None, M_Z2, True, False, par)
                    site_mm(t["z2b"][1], "w1b", (128, 256), a1h[1], a1h[1],
                            None, M_Z2, False, True, par)
                    yield
                    a2 = wk.tile([128, 2, RG], a2dt, tag=f"a2_{gi}")
                    if not fb01:
                        ACT(a2[:], t["z2"], Act.Tanh)
                    else:
                        ACT(a2[:, 0, :], t["z2b"][0], Act.Tanh,
                            bias=bias[:, 2:3])
                        ACT(a2[:, 1, :], t["z2b"][1], Act.Tanh,
                            bias=bias[:, 3:4])
                    yield
                    a2h = [a2[:, 0, :], a2[:, 1, :]]
                    if not last:
                        fin = k == K - 2
                        site_mm(t["z1b"][0], "W20a", (0, 128), a2h[0],
                                a2h[0], None, M_W20, False, False, par)
                        site_mm(t["z1b"][0], "W20b", (0, 128), a2h[1],
                                a2h[1], None, M_W20, False, fin, par)
                        site_mm(t["z1b"][1], "W20a", (128, 256), a2h[0],
                                a2h[0], None, M_W20, False, False, par)
                        site_mm(t["z1b"][1], "W20b", (128, 256), a2h[1],
                                a2h[1], None, M_W20, False, fin, par)
                    site_mm(t["hd"], "w2a", (0, 128), a2h[0], a2h[0], None,
                            M_W20, k == 0, False, par)
                    site_mm(t["hd"], "w2b", (0, 128), a2h[1], a2h[1], None,
                            M_W20, False, last, par)
                    yield

                # h1 = h_prev + (hd + DT*b2); the copy consumed by the
                # whh0 chain matmuls is emitted FIRST on DVE
                h1h = hist[:, s, gi * RG:(gi + 1) * RG]
                h1f = st.tile([128, RG], f32, tag=f"h1f_{gi}")
                stts = [h1h, h1f[:]]
                if M_G0H == "f32":
                    stts.reverse()
                for dst in stts:
                    V.scalar_tensor_tensor(dst, t["hd"], bias[:, 4:5], hpf,
                                           op0=Alu.add, op1=Alu.add)
                h1l = None
                if need_h1l:
                    h1lt = wk.tile([128, RG], f16, tag=f"h1l_{gi}")
                    GP.tensor_sub(h1lt[:], h1f[:], h1h)
                    h1l = h1lt[:]
                yield

                # GRU0 matmuls + GRU1 g-gate hidden side + wout projection
                MM(t["pg0_ig"], W["wih6"][:, 256:384], xsl, start=True,
                   stop=True)
                # dep-free x-openers first (the two prz0 blocks live in
                # different banks, so both groups may be open at once);
                # the h1f-gated whh0 closers run back-to-back after
                MM(t["prz0b"][0], W["wih6"][:, 0:128], xsl, start=True,
                   stop=False)
                MM(t["prz0b"][1], W["wih6"][:, 128:256], xsl, start=True,
                   stop=False)
                site_mm(t["prz0b"][0], "whh0", (0, 128), h1f[:], h1h, h1l,
                        M_G0H, False, True)
                site_mm(t["prz0b"][1], "whh0", (128, 256), h1f[:], h1h, h1l,
                        M_G0H, False, True)
                site_mm(t["pg0_hg"], "whh0", (256, 384), h1f[:], h1h, h1l,
                        M_G0H, True, True)
                site_mm(t["pg1_hg"], "whh1", (256, 384), h1f[:], h1h, h1l,
                        M_G1H, True, True)
                MM(t["py"], W["wouth"][:], h1h, start=True, stop=True)
                # open GRU1 r/z groups on the h1-dependent halves now; the
                # wih1 input-side terms arrive as v0/p0 products later
                # (h2 = u0*n0 + zz0*h1 distributed through the PE)
                site_mm(t["prz1b"][0], "whh1", (0, 128), h1f[:], h1h, h1l,
                        M_G1H, True, False)
                site_mm(t["prz1b"][1], "whh1", (128, 256), h1f[:], h1h, h1l,
                        M_G1H, True, False)
                yield

                rz0 = wk.tile([128, 2, RG], f32, tag=f"rz0_{gi}")
                if not frz0:
                    ACT(rz0[:], t["prz0"], Act.Sigmoid)
                else:
                    ACT(rz0[:, 0, :], t["prz0b"][0], Act.Sigmoid,
                        bias=bias[:, 5:6])
                    ACT(rz0[:, 1, :], t["prz0b"][1], Act.Sigmoid,
                        bias=bias[:, 6:7])
                r0, zz0 = rz0[:, 0, :], rz0[:, 1, :]
                yield

                hg0 = t["pg0_hg"]
                if fg0h:
                    hg0t = wk.tile([128, RG], f32, tag=f"hg0t_{gi}")
                    nc.scalar.add(hg0t[:], hg0, bias[:, 8:9])
                    hg0 = hg0t[:]
                # chain: s1 -> np0 -> [n0]; u0/v precomputed for the tail
                s1 = wk.tile([128, RG], f32, tag=f"s1_{gi}")
                V.tensor_mul(s1[:], r0, hg0)
                np0 = wk.tile([128, RG], f32, tag=f"np0_{gi}")
                V.tensor_add(np0[:], s1[:], t["pg0_ig"])
                u0 = wk.tile([128, RG], f32, tag=f"u0_{gi}")
                V.tensor_scalar(u0[:], zz0, -1.0, 1.0, op0=Alu.mult,
                                op1=Alu.add)
                g1dt = f32 if M_G1I == "f32" else f16
                v0 = wk.tile([128, RG], g1dt, tag=f"v0_{gi}")
                V.tensor_mul(v0[:], zz0, h1f[:])
                yield

                n0 = wk.tile([128, RG], f32, tag=f"n0_{gi}")
                if not fg0i:
                    ACT(n0[:], np0[:], Act.Tanh)
                else:
                    ACT(n0[:], np0[:], Act.Tanh, bias=bias[:, 7:8])
                # v0-half of the GRU1 input-side products
                site_mm(t["prz1b"][0], "wih1", (0, 128), v0[:], v0[:],
                        None, M_G1I, False, False)
                site_mm(t["prz1b"][1], "wih1", (128, 256), v0[:], v0[:],
                        None, M_G1I, False, False)
                site_mm(t["pg1_ig"], "wih1", (256, 384), v0[:], v0[:],
                        None, M_G1I, True, False)
                yield

                # only p0 = u0*n0 sits on the chain after n0
                p0 = wk.tile([128, RG], g1dt, tag=f"p0_{gi}")
                V.tensor_mul(p0[:], u0[:], n0[:])
                # y slice out (off critical path, backfills this phase)
                ysl = wk.tile([DOUT, RG], f32, tag=f"ysl_{gi}")
                if not fbout:
                    V.tensor_copy(ysl[:], t["py"])
                else:
                    ACT(ysl[:], t["py"], Act.Identity,
                        bias=bias[0:DOUT, 15:16])
                nc.sync.dma_start(out=d_y[:, c0:c1], in_=ysl[:])
                yield

                # close GRU1 groups with the p0-half products
                site_mm(t["prz1b"][0], "wih1", (0, 128), p0[:], p0[:],
                        None, M_G1I, False, True)
                site_mm(t["prz1b"][1], "wih1", (128, 256), p0[:], p0[:],
                        None, M_G1I, False, True)
                site_mm(t["pg1_ig"], "wih1", (256, 384), p0[:], p0[:],
                        None, M_G1I, False, True)
                yield

                rz1 = wk.tile([128, 2, RG], f32, tag=f"rz1_{gi}")
                if not frz1:
                    ACT(rz1[:], t["prz1"], Act.Sigmoid)
                else:
                    ACT(rz1[:, 0, :], t["prz1b"][0], Act.Sigmoid,
                        bias=bias[:, 9:10])
                    ACT(rz1[:, 1, :], t["prz1b"][1], Act.Sigmoid,
                        bias=bias[:, 10:11])
                r1, u1 = rz1[:, 0, :], rz1[:, 1, :]
                yield

                hg1 = t["pg1_hg"]
                if fg1h:
                    hg1t = wk.tile([128, RG], f32, tag=f"hg1t_{gi}")
                    nc.scalar.add(hg1t[:], hg1, bias[:, 12:13])
                    hg1 = hg1t[:]
                # chain: s2 -> np1 -> [n1]; gm/gh/q precomputed for the tail
                s2 = wk.tile([128, RG], f32, tag=f"s2_{gi}")
                V.tensor_mul(s2[:], r1, hg1)
                np1 = wk.tile([128, RG], f32, tag=f"np1_{gi}")
                V.tensor_add(np1[:], s2[:], t["pg1_ig"])
                gm = wk.tile([128, RG], f32, tag=f"gm_{gi}")
                V.tensor_mul(gm[:], u1, msl)
                gh = wk.tile([128, RG], f32, tag=f"gh_{gi}")
                V.tensor_mul(gh[:], gm[:], h1f[:])
                q = st.tile([128, RG], f32, tag=f"q_{gi}")
                V.tensor_sub(q[:], h1f[:], gh[:])
                if M_Z1 == "f32":
                    qm = q
                else:
                    qm = st.tile([128, RG], f16, tag=f"q16_{gi}")
                    V.tensor_copy(qm[:], q[:])
                if s + 1 < NSTEPS:
                    # open next step's z1 telescope early on the q-half
                    tn = slots[((s + 1) * G + gi) % NSLOT]
                    site_mm(tn["z1b"][0], "w0", (0, 128), qm[:], qm[:], None,
                            M_Z1, True, False)
                    site_mm(tn["z1b"][1], "w0", (128, 256), qm[:], qm[:],
                            None, M_Z1, True, False)
                yield

                n1 = wk.tile([128, RG], f32, tag=f"n1_{gi}")
                if not fg1i:
                    ACT(n1[:], np1[:], Act.Tanh)
                else:
                    ACT(n1[:], np1[:], Act.Tanh, bias=bias[:, 11:12])
                yield

                # h_next = h1*(1-g) + g*n1 = q + g*n1, g = mm*u1
                z1dt = f32 if M_Z1 == "f32" else f16
                p1 = st.tile([128, RG], z1dt, tag=f"p1_{gi}")
                V.tensor_mul(p1[:], gm[:], n1[:])
                if s + 1 < NSTEPS:
                    # p1-half of next step's z1 telescope (chain never
                    # touches hn itself)
                    tn = slots[((s + 1) * G + gi) % NSLOT]
                    site_mm(tn["z1b"][0], "w0", (0, 128), p1[:], p1[:],
                            None, M_Z1, False, False)
                    site_mm(tn["z1b"][1], "w0", (128, 256), p1[:], p1[:],
                            None, M_Z1, False, False)
                if M_Z1 == "f32":
                    p1x = p1
                else:
                    p1x = st.tile([128, RG], f32, tag=f"p1x_{gi}")
                    GP.tensor_mul(p1x[:], gm[:], n1[:])
                hn = st.tile([128, RG], f32, tag=f"hn_{gi}")
                V.tensor_add(hn[:], q[:], p1x[:])
                hpf = hn[:]
                yield

        # staggered round-robin phase interleave across groups
        PH = 11 + 4 * K         # yields per step
        STAG = int(os.environ.get("ODERNN_STAG", "20"))
        gens = [group_body(gi) for gi in range(G)]
        started = [False] * G
        done = [False] * G
        tick = 0
        while not all(done):
            for gi in range(G):
                if tick >= gi * STAG and not done[gi]:
                    started[gi] = True
                    try:
                        next(gens[gi])
                    except StopIteration:
                        done[gi] = True
            tick += 1

        pp.release()
        wk.release()
        st.release()
        wp.release()

    nc.compile()
    return nc


def _prep(inputs):
    x2d = np.asarray(inputs["x2d"], np.float32)
    mask = np.asarray(inputs["mask"])
    g = lambda n: np.asarray(inputs[n], np.float32)
    w0, b0 = g("ode_w0"), g("ode_b0")
    w1, b1 = g("ode_w1"), g("ode_b1")
    w2, b2 = g("ode_w2"), g("ode_b2")
    wih0, whh0 = g("wih0"), g("whh0")
    bih0, bhh0 = g("bih0"), g("bhh0")
    wih1, whh1 = g("wih1"), g("whh1")
    bih1, bhh1 = g("bih1"), g("bhh1")
    wout, bout = g("wout"), g("bout")
    h0 = g("h0")

    mf = mask.astype(np.float32)
    xs = (x2d * mf).reshape(B, S, N, DIN)[:, :NSTEPS]
    ms = mf.reshape(B, S, N)[:, :NSTEPS]

    W20 = (DT * (w2.astype(np.float64) @ w0.astype(np.float64))).astype(np.float32)
    h0T = np.repeat(h0.reshape(DRNN, 1), R, axis=1).astype(np.float32)
    h0h, h0l = _split16(h0T)

    whh1n = whh1.copy(); whh1n[:, 128:256] *= -1.0
    wih1n = wih1.copy(); wih1n[:, 128:256] *= -1.0

    bp = np.zeros((DRNN, 24), np.float32)
    bp[:, 0], bp[:, 1] = b0[0:128], b0[128:256]
    bp[:, 2], bp[:, 3] = b1[0:128], b1[128:256]
    bp[:, 4] = DT * b2
    brz0 = bih0 + bhh0
    bp[:, 5], bp[:, 6] = brz0[0:128], brz0[128:256]
    bp[:, 7] = bih0[256:384]
    bp[:, 8] = bhh0[256:384]
    brz1 = bih1 + bhh1
    bp[:, 9] = brz1[0:128]
    bp[:, 10] = -brz1[128:256]          # z-gate negated
    bp[:, 11] = bih1[256:384]
    bp[:, 12] = bhh1[256:384]
    bp[0:DOUT, 15] = bout
    zb = DT * (b2 @ w0)
    for k in range(K):
        bp[:, 16 + 2 * k + 0] = b0[0:128] + k * zb[0:128]
        bp[:, 16 + 2 * k + 1] = b0[128:256] + k * zb[128:256]

    flags = (
        bool(np.any(b0) or np.any(b1) or np.any(b2)),
        bool(np.any(b2)),
        bool(np.any(brz0[0:256])),
        bool(np.any(bhh0[256:384])),
        bool(np.any(bih0[256:384])),
        bool(np.any(brz1[0:256])),
        bool(np.any(bhh1[256:384])),
        bool(np.any(bih1[256:384])),
        bool(np.any(bout)),
    )

    C = np.ascontiguousarray

    def wpack(name, arr, mode):
        out = {}
        if mode in ("f16x2w", "f16x3"):
            hi, lo = _split16(arr)
            out[name + "h"] = C(hi)
            out[name + "l"] = C(lo)
        elif mode == "f16":
            out[name] = C(arr.astype(F16))
        elif mode == "f16d":
            nv = 3 if name.startswith("W20") else 4
            for j, v in enumerate(_dither16(arr, nv)):
                out[f"{name}d{j}"] = C(v)
        else:
            out[name] = C(arr.astype(np.float32))
        return out

    shared = {}
    shared.update(wpack("w0", w0, M_Z1))
    shared.update(wpack("w1a", w1[0:128], M_Z2))
    shared.update(wpack("w1b", w1[128:256], M_Z2))
    shared.update(wpack("w2a", DT * w2[0:128], M_W20))
    shared.update(wpack("w2b", DT * w2[128:256], M_W20))
    shared.update(wpack("W20a", W20[0:128], M_W20))
    shared.update(wpack("W20b", W20[128:256], M_W20))
    shared.update(wpack("whh0", whh0, M_G0H))
    shared.update(wpack("whh1", whh1n, M_G1H))
    shared.update(wpack("wih1", wih1n, M_G1I))
    wih0h, wih0l = _split16(wih0)
    shared["wih6"] = C(np.concatenate([wih0h, wih0h, wih0l], axis=0))
    shared["wouth"] = C(wout.astype(F16))
    shared["biaspk"] = bp
    shared["h0f"] = h0T
    shared["h0h"] = C(h0h)
    shared["h0l"] = C(h0l)

    in_maps = []
    for c in range(NCORES):
        xc = xs[c * BL:(c + 1) * BL]
        xmT = xc.transpose(3, 1, 0, 2).reshape(DIN, SR)
        xh, xl = _split16(xmT)
        xm6 = np.concatenate([xh, xl, xh], axis=0)
        mc = ms[c * BL:(c + 1) * BL]
        mrow = mc.transpose(1, 0, 2).reshape(1, SR)
        mbc = np.broadcast_to(mrow, (DRNN, SR)).astype(F16)
        m = dict(shared)
        m["xm6"] = C(xm6)
        m["mm16"] = C(mbc)
        in_maps.append(m)
    return in_maps, flags


def kernel(**inputs):
    in_maps, flags = _prep(inputs)
    if flags not in _prog_cache:
        _prog_cache[flags] = _build_program(flags)
    nc = _prog_cache[flags]

    from concourse.bass_utils import run_bass_kernel_spmd
    res = run_bass_kernel_spmd(nc, in_maps, core_ids=list(range(NCORES)))
    global _last_results
    _last_results = res.results

    ys = np.zeros((B, NSTEPS, P, J, DOUT), np.float32)
    for c in range(NCORES):
        y = res.results[c]["y"]                      # (DOUT, SR)
        y = y.reshape(DOUT, NSTEPS, BL, N).transpose(2, 1, 3, 0)
        ys[c * BL:(c + 1) * BL] = y.reshape(BL, NSTEPS, P, J, DOUT)
    return ys



# revision 40
# speedup vs baseline: 1.0079x; 1.0077x over previous
"""ODE-RNN Trainium2 Bass kernel — v3 (fp16 matmuls, skewed group pipeline,
PE-distributed GRU algebra).

Data-parallel over batch across 8 NeuronCores (4 batches x 34 slots = 136
rows/core).  State kept transposed [DRNN=128 partitions, rows on free dim].
Matmuls run in fp16 (1 cyc/row vs 4 for fp32 on the PE) with per-site
precision modes:
  f16    -- both operands fp16 (one matmul)
  f16x2w -- W split hi/lo fp16, moving single fp16 (2 matmuls)
  f16x3  -- W and moving both split hi/lo fp16 (3 matmuls, ~fp32)
  f32    -- full fp32 (4 cyc/row)
ODE Euler steps telescoped through PSUM accumulation (z1 += DT*w2w0^T a2).
GRU1 z-gate weights negated host-side so sigmoid directly yields 1-z.
Vector-op chain hops are traded for PSUM-accumulated matmuls:
  wih1^T h2   = wih1^T (zz0*h1) [early] + wih1^T (u0*n0) [post-n0]
  w0^T h_next = w0^T q [pre-n1] + w0^T p1 [post-n1],  q = h1*(1-g), p1 = g*n1
so the dependency chain never waits on h2 or h_next materialization.

Scheduling: engine SEQs are strict FIFO; dependency waits are blocking
EventSemaphores.  The G row-groups are emitted PHASE-INTERLEAVED with a
stagger of PH/G phases so that while group 0 waits on its activation,
group 1's matmuls (already dependency-satisfied) sit next in the stream.

PSUM (8 banks of 2KB): per group-slot 4 banks; long-open accumulation
groups (z1 blocks, hd) each own a bank, quick open/close pairs share
banks strictly back-to-back (HW-verified invariant from v1).
"""

import os
import numpy as np
import ml_dtypes

B, S, P, J = 32, 128, 2, 17
DIN, DOUT, DRNN, DHID = 2, 3, 128, 256
N = P * J            # 34
DT = 0.1
K = 4                # Euler steps
NCORES = 8
BL = B // NCORES     # 4 batches per core
R = BL * N           # 136 rows per core

G = int(os.environ.get("ODERNN_G", "2"))
NSTEPS = int(os.environ.get("ODERNN_STEPS", S))
SR = NSTEPS * R

M_Z2 = os.environ.get("ODERNN_M_Z2", "f16x2w")   # w1^T a1  (undamped)
M_W20 = os.environ.get("ODERNN_M_W20", "f16x2w") # W20/w2^T a2 (DT-damped)
M_Z1 = os.environ.get("ODERNN_M_Z1", "f32")      # w0^T h   (z1 telescope)
M_G0H = os.environ.get("ODERNN_M_G0H", "f32")    # whh0^T h1
M_G1H = os.environ.get("ODERNN_M_G1H", "f32")    # whh1^T h1 (off-chain)
M_G1I = os.environ.get("ODERNN_M_G1I", "f32")    # wih1^T h2

F16 = np.float16

_prog_cache = {}


def _split16(a):
    hi = a.astype(F16)
    lo = (a.astype(np.float32) - hi.astype(np.float32)).astype(F16)
    return hi, lo


_DITHER_PATTERNS = {
    3: {0: "qqq", 1: "qpq", 2: "pqp", 3: "ppp"},
    4: {0: "qqqq", 1: "qpqq", 2: "qpqp", 3: "pqpp", 4: "pppp"},
}


def _dither16(a, phases=4):
    """`phases` f16 tensors whose per-element duty-cycled average best
    approximates a, with slot patterns that also cancel linear drift of
    the moving operand across the cycle."""
    a = a.astype(np.float64)
    p = a.astype(F16)
    pf = p.astype(np.float64)
    # neighbor on the far side of a (or equal when exact)
    toward = np.where(a >= pf, np.float16(np.inf), np.float16(-np.inf))
    q = np.nextafter(p, toward.astype(F16))
    qf = q.astype(np.float64)
    ns = np.arange(phases + 1).reshape((-1,) + (1,) * a.ndim)
    means = (ns * pf + (phases - ns) * qf) / phases
    pick = np.argmin(np.abs(means - a), axis=0)    # n_p per element
    pats = _DITHER_PATTERNS[phases]
    outs = []
    for j in range(phases):
        use_p = np.zeros(a.shape, bool)
        for n, pat in pats.items():
            use_p |= (pick == n) & (pat[j] == "p")
        outs.append(np.where(use_p, p, q).astype(F16))
    return outs


def _build_program(flags):
    import concourse.tile as tile
    import concourse.mybir as mybir
    from concourse import bacc

    (fb01, fb2, frz0, fg0h, fg0i, frz1, fg1h, fg1i, fbout) = flags

    dt = mybir.dt
    f32 = dt.float32
    f16 = dt.float16
    Alu = mybir.AluOpType
    Act = mybir.ActivationFunctionType

    nc = bacc.Bacc("TRN2", target_bir_lowering=False)

    RG = R // G
    assert R % G == 0 and RG <= 120

    def wdt(mode):
        return f32 if mode == "f32" else f16

    # ---- DRAM I/O ----
    d_xm6 = nc.dram_tensor("xm6", [6, SR], f16, kind="ExternalInput")
    d_mm = nc.dram_tensor("mm16", [DRNN, SR], f16, kind="ExternalInput")
    d_h0f = nc.dram_tensor("h0f", [DRNN, R], f32, kind="ExternalInput")
    d_h0h = nc.dram_tensor("h0h", [DRNN, R], f16, kind="ExternalInput")
    d_h0l = nc.dram_tensor("h0l", [DRNN, R], f16, kind="ExternalInput")

    dram_w = {}

    def wdecl(name, shape, mode):
        if mode in ("f16x2w", "f16x3"):
            dram_w[name + "h"] = nc.dram_tensor(name + "h", shape, f16,
                                                kind="ExternalInput")
            dram_w[name + "l"] = nc.dram_tensor(name + "l", shape, f16,
                                                kind="ExternalInput")
        elif mode == "f16d":
            nv = 3 if name.startswith("W20") else 4
            for j in range(nv):
                dram_w[f"{name}d{j}"] = nc.dram_tensor(
                    f"{name}d{j}", shape, f16, kind="ExternalInput")
        else:
            dram_w[name] = nc.dram_tensor(name, shape, wdt(mode),
                                          kind="ExternalInput")

    wdecl("w0", [DRNN, DHID], M_Z1)
    wdecl("w1a", [128, DHID], M_Z2)
    wdecl("w1b", [128, DHID], M_Z2)
    wdecl("w2a", [128, DRNN], M_W20)
    wdecl("w2b", [128, DRNN], M_W20)
    wdecl("W20a", [128, DHID], M_W20)
    wdecl("W20b", [128, DHID], M_W20)
    wdecl("whh0", [DRNN, 3 * DRNN], M_G0H)
    wdecl("whh1", [DRNN, 3 * DRNN], M_G1H)   # z-gate cols negated
    wdecl("wih1", [DRNN, 3 * DRNN], M_G1I)   # z-gate cols negated
    dram_w["wih6"] = nc.dram_tensor("wih6", [6, 3 * DRNN], f16,
                                    kind="ExternalInput")
    dram_w["wouth"] = nc.dram_tensor("wouth", [DRNN, DOUT], f16,
                                     kind="ExternalInput")
    d_bias = nc.dram_tensor("biaspk", [DRNN, 24], f32, kind="ExternalInput")
    d_y = nc.dram_tensor("y", [DOUT, SR], f32, kind="ExternalOutput")

    with tile.TileContext(nc) as tc:
        wp = tc.alloc_tile_pool(name="wconst", bufs=1)
        st = tc.alloc_tile_pool(name="state", bufs=4)
        wk = tc.alloc_tile_pool(name="work", bufs=6)
        pp = tc.alloc_tile_pool(name="psum", bufs=1, space="PSUM")

        def load(dram, shape, dtype, name):
            t = wp.tile(shape, dtype, tag=name, name=name)
            nc.sync.dma_start(out=t[:], in_=dram[:])
            return t

        W = {}
        for nm, dten in dram_w.items():
            W[nm] = load(dten, list(dten.shape), dten.dtype, nm)
        xm6 = load(d_xm6, [6, SR], f16, "xm6")
        h0f = load(d_h0f, [DRNN, R], f32, "h0f")
        h0h = load(d_h0h, [DRNN, R], f16, "h0h")
        bias = load(d_bias, [DRNN, 24], f32, "biaspk")
        need_hl = M_Z1 == "f16x3"
        need_h1l = "f16x3" in (M_G0H, M_G1H)
        h0l = load(d_h0l, [DRNN, R], f16, "h0l") if need_hl else None
        mm16 = load(d_mm, [DRNN, SR], f16, "mm16")

        hist = wp.tile([DRNN, NSTEPS, R], f16, tag="hist", name="hist")

        MM = nc.tensor.matmul
        ACT = nc.scalar.activation
        V = nc.vector
        GP = nc.gpsimd if os.environ.get("ODERNN_GP", "1") == "1" else nc.vector
        _gpops = set(os.environ.get("ODERNN_GPOPS", "ysl,q").split(","))

        def ENG(name):
            return nc.gpsimd if name in _gpops else nc.vector

        # ---- PSUM: NSLOT slots x {tz1 (2 banks), tAB (2 banks)} ----
        # Long-open groups (z1 blocks, hd) own their banks; quick pairs
        # share strictly back-to-back.  prz1 blocks live in SEPARATE banks
        # (same region index of tAB's two banks) so both can be opened
        # early by the whh1 matmuls while staying one-open-per-bank, and
        # the merged rz1 ACT still sees one strided AP.
        NSLOT = 2
        slots = []
        for si in range(NSLOT):
            tz1 = pp.tile([128, 2, 512], f32, tag=f"tz1_{si}",
                          name=f"tz1_{si}")
            tAB = pp.tile([128, 2, 4, 128], f32, tag=f"tAB_{si}",
                          name=f"tAB_{si}")
            slots.append({
                "z1": tz1[:, :, 0:RG],
                "z1b": [tz1[:, 0, 0:RG], tz1[:, 1, 0:RG]],
                "prz0": tz1[:, :, 128:128 + RG],
                "prz0b": [tz1[:, 0, 128:128 + RG], tz1[:, 1, 128:128 + RG]],
                "pg0_ig": tz1[:, 0, 256:256 + RG],
                "pg0_hg": tz1[:, 1, 256:256 + RG],
                "pg1_ig": tz1[:, 0, 384:384 + RG],
                "z2": tAB[:, 0, 0:2, 0:RG],
                "z2b": [tAB[:, 0, 0, 0:RG], tAB[:, 0, 1, 0:RG]],
                "prz1": tAB[:, :, 2, 0:RG],
                "prz1b": [tAB[:, 0, 2, 0:RG], tAB[:, 1, 2, 0:RG]],
                "hd": tAB[:, 1, 0, 0:RG],
                "pg1_hg": tAB[:, 1, 1, 0:RG],
                "py": tAB[0:DOUT, 1, 3, 0:RG],
            })

        def site_mm(out, wname, blk, mov_f32, mov_h, mov_l, mode,
                    start, stop, par=0):
            lo, hi = blk
            if mode == "f32":
                MM(out, W[wname][:, lo:hi], mov_f32, start=start, stop=stop)
            elif mode == "f16":
                MM(out, W[wname][:, lo:hi], mov_h, start=start, stop=stop)
            elif mode == "f16d":
                nv = 3 if wname.startswith("W20") else 4
                MM(out, W[f"{wname}d{par % nv}"][:, lo:hi],
                   mov_h, start=start, stop=stop)
            elif mode == "f16x2w":
                MM(out, W[wname + "h"][:, lo:hi], mov_h, start=start,
                   stop=False)
                MM(out, W[wname + "l"][:, lo:hi], mov_h, start=False,
                   stop=stop)
            else:  # f16x3
                MM(out, W[wname + "h"][:, lo:hi], mov_h, start=start,
                   stop=False)
                MM(out, W[wname + "h"][:, lo:hi], mov_l, start=False,
                   stop=False)
                MM(out, W[wname + "l"][:, lo:hi], mov_h, start=False,
                   stop=stop)

        a1dt = f32 if M_Z2 == "f32" else f16
        a2dt = f32 if M_W20 == "f32" else f16

        def group_body(gi):
            """Generator emitting one group's full sequence; yields at
            phase boundaries for cross-group interleaving."""
            hpf = h0f[:, gi * RG:(gi + 1) * RG]
            hph = h0h[:, gi * RG:(gi + 1) * RG]
            hpl = h0l[:, gi * RG:(gi + 1) * RG] if need_hl else None

            for s in range(NSTEPS):
                t = slots[(s * G + gi) % NSLOT]
                c0 = s * R + gi * RG
                c1 = c0 + RG
                xsl = xm6[:, c0:c1]
                msl = mm16[:, c0:c1]

                # ph0: open z1 telescopes.  For s>0 with M_Z1=f32 this
                # already happened during the previous step's tail:
                # z1 = w0^T h_next = w0^T q + w0^T p1 distributed through
                # the PE, so the chain never waits on an h_next vector op.
                if s == 0:
                    site_mm(t["z1b"][0], "w0", (0, 128), hpf, hph, hpl,
                            M_Z1, True, False)
                    site_mm(t["z1b"][1], "w0", (128, 256), hpf, hph, hpl,
                            M_Z1, True, False)
                yield

                for k in range(K):
                    last = k == K - 1
                    a1 = wk.tile([128, 2, RG], a1dt, tag=f"a1_{gi}")
                    if not fb01:
                        ACT(a1[:], t["z1"], Act.Tanh)
                    else:
                        ACT(a1[:, 0, :], t["z1b"][0], Act.Tanh,
                            bias=bias[:, 16 + 2 * k:17 + 2 * k])
                        ACT(a1[:, 1, :], t["z1b"][1], Act.Tanh,
                            bias=bias[:, 17 + 2 * k:18 + 2 * k])
                    yield
                    a1h = [a1[:, 0, :], a1[:, 1, :]]
                    par = (s * K + k) & 3
                    site_mm(t["z2b"][0], "w1a", (0, 128), a1h[0], a1h[0],
                            None, M_Z2, True, False, par)
                    site_mm(t["z2b"][0], "w1b", (0, 128), a1h[1], a1h[1],
                            None, M_Z2, False, True, par)
                    site_mm(t["z2b"][1], "w1a", (128, 256), a1h[0], a1h[0],
                            None, M_Z2, True, False, par)
                    site_mm(t["z2b"][1], "w1b", (128, 256), a1h[1], a1h[1],
                            None, M_Z2, False, True, par)
                    yield
                    a2 = wk.tile([128, 2, RG], a2dt, tag=f"a2_{gi}")
                    if not fb01:
                        ACT(a2[:], t["z2"], Act.Tanh)
                    else:
                        ACT(a2[:, 0, :], t["z2b"][0], Act.Tanh,
                            bias=bias[:, 2:3])
                        ACT(a2[:, 1, :], t["z2b"][1], Act.Tanh,
                            bias=bias[:, 3:4])
                    yield
                    a2h = [a2[:, 0, :], a2[:, 1, :]]
                    if not last:
                        fin = k == K - 2
                        site_mm(t["z1b"][0], "W20a", (0, 128), a2h[0],
                                a2h[0], None, M_W20, False, False, par)
                        site_mm(t["z1b"][0], "W20b", (0, 128), a2h[1],
                                a2h[1], None, M_W20, False, fin, par)
                        site_mm(t["z1b"][1], "W20a", (128, 256), a2h[0],
                                a2h[0], None, M_W20, False, False, par)
                        site_mm(t["z1b"][1], "W20b", (128, 256), a2h[1],
                                a2h[1], None, M_W20, False, fin, par)
                    site_mm(t["hd"], "w2a", (0, 128), a2h[0], a2h[0], None,
                            M_W20, k == 0, False, par)
                    site_mm(t["hd"], "w2b", (0, 128), a2h[1], a2h[1], None,
                            M_W20, False, last, par)
                    yield

                # h1 = h_prev + (hd + DT*b2); the copy consumed by the
                # whh0 chain matmuls is emitted FIRST on DVE
                h1h = hist[:, s, gi * RG:(gi + 1) * RG]
                h1f = st.tile([128, RG], f32, tag=f"h1f_{gi}")
                stts = [h1h, h1f[:]]
                if M_G0H == "f32":
                    stts.reverse()
                for dst in stts:
                    V.scalar_tensor_tensor(dst, t["hd"], bias[:, 4:5], hpf,
                                           op0=Alu.add, op1=Alu.add)
                h1l = None
                if need_h1l:
                    h1lt = wk.tile([128, RG], f16, tag=f"h1l_{gi}")
                    GP.tensor_sub(h1lt[:], h1f[:], h1h)
                    h1l = h1lt[:]
                yield

                # GRU0 matmuls + GRU1 g-gate hidden side + wout projection
                MM(t["pg0_ig"], W["wih6"][:, 256:384], xsl, start=True,
                   stop=True)
                # dep-free x-openers first (the two prz0 blocks live in
                # different banks, so both groups may be open at once);
                # the h1f-gated whh0 closers run back-to-back after
                MM(t["prz0b"][0], W["wih6"][:, 0:128], xsl, start=True,
                   stop=False)
                MM(t["prz0b"][1], W["wih6"][:, 128:256], xsl, start=True,
                   stop=False)
                site_mm(t["prz0b"][0], "whh0", (0, 128), h1f[:], h1h, h1l,
                        M_G0H, False, True)
                site_mm(t["prz0b"][1], "whh0", (128, 256), h1f[:], h1h, h1l,
                        M_G0H, False, True)
                site_mm(t["pg0_hg"], "whh0", (256, 384), h1f[:], h1h, h1l,
                        M_G0H, True, True)
                site_mm(t["pg1_hg"], "whh1", (256, 384), h1f[:], h1h, h1l,
                        M_G1H, True, True)
                MM(t["py"], W["wouth"][:], h1h, start=True, stop=True)
                # open GRU1 r/z groups on the h1-dependent halves now; the
                # wih1 input-side terms arrive as v0/p0 products later
                # (h2 = u0*n0 + zz0*h1 distributed through the PE)
                site_mm(t["prz1b"][0], "whh1", (0, 128), h1f[:], h1h, h1l,
                        M_G1H, True, False)
                site_mm(t["prz1b"][1], "whh1", (128, 256), h1f[:], h1h, h1l,
                        M_G1H, True, False)
                yield

                rz0 = wk.tile([128, 2, RG], f32, tag=f"rz0_{gi}")
                if not frz0:
                    ACT(rz0[:], t["prz0"], Act.Sigmoid)
                else:
                    ACT(rz0[:, 0, :], t["prz0b"][0], Act.Sigmoid,
                        bias=bias[:, 5:6])
                    ACT(rz0[:, 1, :], t["prz0b"][1], Act.Sigmoid,
                        bias=bias[:, 6:7])
                r0, zz0 = rz0[:, 0, :], rz0[:, 1, :]
                yield

                hg0 = t["pg0_hg"]
                if fg0h:
                    hg0t = wk.tile([128, RG], f32, tag=f"hg0t_{gi}")
                    nc.scalar.add(hg0t[:], hg0, bias[:, 8:9])
                    hg0 = hg0t[:]
                # chain: s1 -> np0 -> [n0]; u0/v precomputed for the tail
                s1 = wk.tile([128, RG], f32, tag=f"s1_{gi}")
                V.tensor_mul(s1[:], r0, hg0)
                np0 = wk.tile([128, RG], f32, tag=f"np0_{gi}")
                V.tensor_add(np0[:], s1[:], t["pg0_ig"])
                u0 = wk.tile([128, RG], f32, tag=f"u0_{gi}")
                ENG("u0").tensor_scalar(u0[:], zz0, -1.0, 1.0, op0=Alu.mult,
                                        op1=Alu.add)
                g1dt = f32 if M_G1I == "f32" else f16
                v0 = wk.tile([128, RG], g1dt, tag=f"v0_{gi}")
                ENG("v0").tensor_mul(v0[:], zz0, h1f[:])
                yield

                n0 = wk.tile([128, RG], f32, tag=f"n0_{gi}")
                if not fg0i:
                    ACT(n0[:], np0[:], Act.Tanh)
                else:
                    ACT(n0[:], np0[:], Act.Tanh, bias=bias[:, 7:8])
                # v0-half of the GRU1 input-side products
                site_mm(t["prz1b"][0], "wih1", (0, 128), v0[:], v0[:],
                        None, M_G1I, False, False)
                site_mm(t["prz1b"][1], "wih1", (128, 256), v0[:], v0[:],
                        None, M_G1I, False, False)
                site_mm(t["pg1_ig"], "wih1", (256, 384), v0[:], v0[:],
                        None, M_G1I, True, False)
                yield

                # only p0 = u0*n0 sits on the chain after n0
                p0 = wk.tile([128, RG], g1dt, tag=f"p0_{gi}")
                ENG("p0").tensor_mul(p0[:], u0[:], n0[:])
                # y slice out (off critical path, backfills this phase)
                ysl = wk.tile([DOUT, RG], f32, tag=f"ysl_{gi}")
                if not fbout:
                    if "ysl" in _gpops:
                        ACT(ysl[:], t["py"], Act.Identity)
                    else:
                        V.tensor_copy(ysl[:], t["py"])
                else:
                    ACT(ysl[:], t["py"], Act.Identity,
                        bias=bias[0:DOUT, 15:16])
                nc.sync.dma_start(out=d_y[:, c0:c1], in_=ysl[:])
                yield

                # close GRU1 groups with the p0-half products
                site_mm(t["prz1b"][0], "wih1", (0, 128), p0[:], p0[:],
                        None, M_G1I, False, True)
                site_mm(t["prz1b"][1], "wih1", (128, 256), p0[:], p0[:],
                        None, M_G1I, False, True)
                site_mm(t["pg1_ig"], "wih1", (256, 384), p0[:], p0[:],
                        None, M_G1I, False, True)
                yield

                rz1 = wk.tile([128, 2, RG], f32, tag=f"rz1_{gi}")
                if not frz1:
                    ACT(rz1[:], t["prz1"], Act.Sigmoid)
                else:
                    ACT(rz1[:, 0, :], t["prz1b"][0], Act.Sigmoid,
                        bias=bias[:, 9:10])
                    ACT(rz1[:, 1, :], t["prz1b"][1], Act.Sigmoid,
                        bias=bias[:, 10:11])
                r1, u1 = rz1[:, 0, :], rz1[:, 1, :]
                yield

                hg1 = t["pg1_hg"]
                if fg1h:
                    hg1t = wk.tile([128, RG], f32, tag=f"hg1t_{gi}")
                    nc.scalar.add(hg1t[:], hg1, bias[:, 12:13])
                    hg1 = hg1t[:]
                # chain: s2 -> np1 -> [n1]; gm/gh/q precomputed for the tail
                s2 = wk.tile([128, RG], f32, tag=f"s2_{gi}")
                V.tensor_mul(s2[:], r1, hg1)
                np1 = wk.tile([128, RG], f32, tag=f"np1_{gi}")
                V.tensor_add(np1[:], s2[:], t["pg1_ig"])
                gm = wk.tile([128, RG], f32, tag=f"gm_{gi}")
                ENG("gm").tensor_mul(gm[:], u1, msl)
                gh = wk.tile([128, RG], f32, tag=f"gh_{gi}")
                ENG("gh").tensor_mul(gh[:], gm[:], h1f[:])
                q = st.tile([128, RG], f32, tag=f"q_{gi}")
                ENG("q").tensor_sub(q[:], h1f[:], gh[:])
                if M_Z1 == "f32":
                    qm = q
                else:
                    qm = st.tile([128, RG], f16, tag=f"q16_{gi}")
                    V.tensor_copy(qm[:], q[:])
                if s + 1 < NSTEPS:
                    # open next step's z1 telescope early on the q-half
                    tn = slots[((s + 1) * G + gi) % NSLOT]
                    site_mm(tn["z1b"][0], "w0", (0, 128), qm[:], qm[:], None,
                            M_Z1, True, False)
                    site_mm(tn["z1b"][1], "w0", (128, 256), qm[:], qm[:],
                            None, M_Z1, True, False)
                yield

                n1 = wk.tile([128, RG], f32, tag=f"n1_{gi}")
                if not fg1i:
                    ACT(n1[:], np1[:], Act.Tanh)
                else:
                    ACT(n1[:], np1[:], Act.Tanh, bias=bias[:, 11:12])
                yield

                # h_next = h1*(1-g) + g*n1 = q + g*n1, g = mm*u1
                z1dt = f32 if M_Z1 == "f32" else f16
                p1 = st.tile([128, RG], z1dt, tag=f"p1_{gi}")
                ENG("p1").tensor_mul(p1[:], gm[:], n1[:])
                if s + 1 < NSTEPS:
                    # p1-half of next step's z1 telescope (chain never
                    # touches hn itself)
                    tn = slots[((s + 1) * G + gi) % NSLOT]
                    site_mm(tn["z1b"][0], "w0", (0, 128), p1[:], p1[:],
                            None, M_Z1, False, False)
                    site_mm(tn["z1b"][1], "w0", (128, 256), p1[:], p1[:],
                            None, M_Z1, False, False)
                if M_Z1 == "f32":
                    p1x = p1
                else:
                    p1x = st.tile([128, RG], f32, tag=f"p1x_{gi}")
                    GP.tensor_mul(p1x[:], gm[:], n1[:])
                hn = st.tile([128, RG], f32, tag=f"hn_{gi}")
                ENG("hn").tensor_add(hn[:], q[:], p1x[:])
                hpf = hn[:]
                yield

        # staggered round-robin phase interleave across groups
        PH = 11 + 4 * K         # yields per step
        STAG = int(os.environ.get("ODERNN_STAG", "20"))
        gens = [group_body(gi) for gi in range(G)]
        started = [False] * G
        done = [False] * G
        tick = 0
        while not all(done):
            for gi in range(G):
                if tick >= gi * STAG and not done[gi]:
                    started[gi] = True
                    try:
                        next(gens[gi])
                    except StopIteration:
                        done[gi] = True
            tick += 1

        pp.release()
        wk.release()
        st.release()
        wp.release()

    nc.compile()
    return nc


def _prep(inputs):
    x2d = np.asarray(inputs["x2d"], np.float32)
    mask = np.asarray(inputs["mask"])
    g = lambda n: np.asarray(inputs[n], np.float32)
    w0, b0 = g("ode_w0"), g("ode_b0")
    w1, b1 = g("ode_w1"), g("ode_b1")
    w2, b2 = g("ode_w2"), g("ode_b2")
    wih0, whh0 = g("wih0"), g("whh0")
    bih0, bhh0 = g("bih0"), g("bhh0")
    wih1, whh1 = g("wih1"), g("whh1")
    bih1, bhh1 = g("bih1"), g("bhh1")
    wout, bout = g("wout"), g("bout")
    h0 = g("h0")

    mf = mask.astype(np.float32)
    xs = (x2d * mf).reshape(B, S, N, DIN)[:, :NSTEPS]
    ms = mf.reshape(B, S, N)[:, :NSTEPS]

    W20 = (DT * (w2.astype(np.float64) @ w0.astype(np.float64))).astype(np.float32)
    h0T = np.repeat(h0.reshape(DRNN, 1), R, axis=1).astype(np.float32)
    h0h, h0l = _split16(h0T)

    whh1n = whh1.copy(); whh1n[:, 128:256] *= -1.0
    wih1n = wih1.copy(); wih1n[:, 128:256] *= -1.0

    bp = np.zeros((DRNN, 24), np.float32)
    bp[:, 0], bp[:, 1] = b0[0:128], b0[128:256]
    bp[:, 2], bp[:, 3] = b1[0:128], b1[128:256]
    bp[:, 4] = DT * b2
    brz0 = bih0 + bhh0
    bp[:, 5], bp[:, 6] = brz0[0:128], brz0[128:256]
    bp[:, 7] = bih0[256:384]
    bp[:, 8] = bhh0[256:384]
    brz1 = bih1 + bhh1
    bp[:, 9] = brz1[0:128]
    bp[:, 10] = -brz1[128:256]          # z-gate negated
    bp[:, 11] = bih1[256:384]
    bp[:, 12] = bhh1[256:384]
    bp[0:DOUT, 15] = bout
    zb = DT * (b2 @ w0)
    for k in range(K):
        bp[:, 16 + 2 * k + 0] = b0[0:128] + k * zb[0:128]
        bp[:, 16 + 2 * k + 1] = b0[128:256] + k * zb[128:256]

    flags = (
        bool(np.any(b0) or np.any(b1) or np.any(b2)),
        bool(np.any(b2)),
        bool(np.any(brz0[0:256])),
        bool(np.any(bhh0[256:384])),
        bool(np.any(bih0[256:384])),
        bool(np.any(brz1[0:256])),
        bool(np.any(bhh1[256:384])),
        bool(np.any(bih1[256:384])),
        bool(np.any(bout)),
    )

    C = np.ascontiguousarray

    def wpack(name, arr, mode):
        out = {}
        if mode in ("f16x2w", "f16x3"):
            hi, lo = _split16(arr)
            out[name + "h"] = C(hi)
            out[name + "l"] = C(lo)
        elif mode == "f16":
            out[name] = C(arr.astype(F16))
        elif mode == "f16d":
            nv = 3 if name.startswith("W20") else 4
            for j, v in enumerate(_dither16(arr, nv)):
                out[f"{name}d{j}"] = C(v)
        else:
            out[name] = C(arr.astype(np.float32))
        return out

    shared = {}
    shared.update(wpack("w0", w0, M_Z1))
    shared.update(wpack("w1a", w1[0:128], M_Z2))
    shared.update(wpack("w1b", w1[128:256], M_Z2))
    shared.update(wpack("w2a", DT * w2[0:128], M_W20))
    shared.update(wpack("w2b", DT * w2[128:256], M_W20))
    shared.update(wpack("W20a", W20[0:128], M_W20))
    shared.update(wpack("W20b", W20[128:256], M_W20))
    shared.update(wpack("whh0", whh0, M_G0H))
    shared.update(wpack("whh1", whh1n, M_G1H))
    shared.update(wpack("wih1", wih1n, M_G1I))
    wih0h, wih0l = _split16(wih0)
    shared["wih6"] = C(np.concatenate([wih0h, wih0h, wih0l], axis=0))
    shared["wouth"] = C(wout.astype(F16))
    shared["biaspk"] = bp
    shared["h0f"] = h0T
    shared["h0h"] = C(h0h)
    shared["h0l"] = C(h0l)

    in_maps = []
    for c in range(NCORES):
        xc = xs[c * BL:(c + 1) * BL]
        xmT = xc.transpose(3, 1, 0, 2).reshape(DIN, SR)
        xh, xl = _split16(xmT)
        xm6 = np.concatenate([xh, xl, xh], axis=0)
        mc = ms[c * BL:(c + 1) * BL]
        mrow = mc.transpose(1, 0, 2).reshape(1, SR)
        mbc = np.broadcast_to(mrow, (DRNN, SR)).astype(F16)
        m = dict(shared)
        m["xm6"] = C(xm6)
        m["mm16"] = C(mbc)
        in_maps.append(m)
    return in_maps, flags


def kernel(**inputs):
    in_maps, flags = _prep(inputs)
    if flags not in _prog_cache:
        _prog_cache[flags] = _build_program(flags)
    nc = _prog_cache[flags]

    from concourse.bass_utils import run_bass_kernel_spmd
    res = run_bass_kernel_spmd(nc, in_maps, core_ids=list(range(NCORES)))
    global _last_results
    _last_results = res.results

    ys = np.zeros((B, NSTEPS, P, J, DOUT), np.float32)
    for c in range(NCORES):
        y = res.results[c]["y"]                      # (DOUT, SR)
        y = y.reshape(DOUT, NSTEPS, BL, N).transpose(2, 1, 3, 0)
        ys[c * BL:(c + 1) * BL] = y.reshape(BL, NSTEPS, P, J, DOUT)
    return ys



# revision 42
# speedup vs baseline: 1.0111x; 1.0032x over previous
"""ODE-RNN Trainium2 Bass kernel — v3 (fp16 matmuls, skewed group pipeline,
PE-distributed GRU algebra).

Data-parallel over batch across 8 NeuronCores (4 batches x 34 slots = 136
rows/core).  State kept transposed [DRNN=128 partitions, rows on free dim].
Matmuls run in fp16 (1 cyc/row vs 4 for fp32 on the PE) with per-site
precision modes:
  f16    -- both operands fp16 (one matmul)
  f16x2w -- W split hi/lo fp16, moving single fp16 (2 matmuls)
  f16x3  -- W and moving both split hi/lo fp16 (3 matmuls, ~fp32)
  f32    -- full fp32 (4 cyc/row)
ODE Euler steps telescoped through PSUM accumulation (z1 += DT*w2w0^T a2).
GRU1 z-gate weights negated host-side so sigmoid directly yields 1-z.
Vector-op chain hops are traded for PSUM-accumulated matmuls:
  wih1^T h2   = wih1^T (zz0*h1) [early] + wih1^T (u0*n0) [post-n0]
  w0^T h_next = w0^T q [pre-n1] + w0^T p1 [post-n1],  q = h1*(1-g), p1 = g*n1
so the dependency chain never waits on h2 or h_next materialization.

Scheduling: engine SEQs are strict FIFO; dependency waits are blocking
EventSemaphores.  The G row-groups are emitted PHASE-INTERLEAVED with a
stagger of PH/G phases so that while group 0 waits on its activation,
group 1's matmuls (already dependency-satisfied) sit next in the stream.

PSUM (8 banks of 2KB): per group-slot 4 banks; long-open accumulation
groups (z1 blocks, hd) each own a bank, quick open/close pairs share
banks strictly back-to-back (HW-verified invariant from v1).
"""

import os
import numpy as np
import ml_dtypes

B, S, P, J = 32, 128, 2, 17
DIN, DOUT, DRNN, DHID = 2, 3, 128, 256
N = P * J            # 34
DT = 0.1
K = 4                # Euler steps
NCORES = 8
BL = B // NCORES     # 4 batches per core
R = BL * N           # 136 rows per core

G = int(os.environ.get("ODERNN_G", "2"))
NSTEPS = int(os.environ.get("ODERNN_STEPS", S))
SR = NSTEPS * R

M_Z2 = os.environ.get("ODERNN_M_Z2", "f16x2w")   # w1^T a1  (undamped)
M_W20 = os.environ.get("ODERNN_M_W20", "f16x2w") # W20/w2^T a2 (DT-damped)
M_Z1 = os.environ.get("ODERNN_M_Z1", "f32")      # w0^T h   (z1 telescope)
M_G0H = os.environ.get("ODERNN_M_G0H", "f32")    # whh0^T h1
M_G1H = os.environ.get("ODERNN_M_G1H", "f32")    # whh1^T h1 (off-chain)
M_G1I = os.environ.get("ODERNN_M_G1I", "f32")    # wih1^T h2

F16 = np.float16

_prog_cache = {}


def _split16(a):
    hi = a.astype(F16)
    lo = (a.astype(np.float32) - hi.astype(np.float32)).astype(F16)
    return hi, lo


_DITHER_PATTERNS = {
    3: {0: "qqq", 1: "qpq", 2: "pqp", 3: "ppp"},
    4: {0: "qqqq", 1: "qpqq", 2: "qpqp", 3: "pqpp", 4: "pppp"},
}


def _dither16(a, phases=4):
    """`phases` f16 tensors whose per-element duty-cycled average best
    approximates a, with slot patterns that also cancel linear drift of
    the moving operand across the cycle."""
    a = a.astype(np.float64)
    p = a.astype(F16)
    pf = p.astype(np.float64)
    # neighbor on the far side of a (or equal when exact)
    toward = np.where(a >= pf, np.float16(np.inf), np.float16(-np.inf))
    q = np.nextafter(p, toward.astype(F16))
    qf = q.astype(np.float64)
    ns = np.arange(phases + 1).reshape((-1,) + (1,) * a.ndim)
    means = (ns * pf + (phases - ns) * qf) / phases
    pick = np.argmin(np.abs(means - a), axis=0)    # n_p per element
    pats = _DITHER_PATTERNS[phases]
    outs = []
    for j in range(phases):
        use_p = np.zeros(a.shape, bool)
        for n, pat in pats.items():
            use_p |= (pick == n) & (pat[j] == "p")
        outs.append(np.where(use_p, p, q).astype(F16))
    return outs


def _build_program(flags):
    import concourse.tile as tile
    import concourse.mybir as mybir
    from concourse import bacc

    (fb01, fb2, frz0, fg0h, fg0i, frz1, fg1h, fg1i, fbout) = flags

    dt = mybir.dt
    f32 = dt.float32
    f16 = dt.float16
    Alu = mybir.AluOpType
    Act = mybir.ActivationFunctionType

    nc = bacc.Bacc("TRN2", target_bir_lowering=False)

    RG = R // G
    assert R % G == 0 and RG <= 120

    def wdt(mode):
        return f32 if mode == "f32" else f16

    # ---- DRAM I/O ----
    d_xm6 = nc.dram_tensor("xm6", [6, SR], f16, kind="ExternalInput")
    d_mm = nc.dram_tensor("mm16", [DRNN, SR], f16, kind="ExternalInput")
    d_h0f = nc.dram_tensor("h0f", [DRNN, R], f32, kind="ExternalInput")
    d_h0h = nc.dram_tensor("h0h", [DRNN, R], f16, kind="ExternalInput")
    d_h0l = nc.dram_tensor("h0l", [DRNN, R], f16, kind="ExternalInput")

    dram_w = {}

    def wdecl(name, shape, mode):
        if mode in ("f16x2w", "f16x3"):
            dram_w[name + "h"] = nc.dram_tensor(name + "h", shape, f16,
                                                kind="ExternalInput")
            dram_w[name + "l"] = nc.dram_tensor(name + "l", shape, f16,
                                                kind="ExternalInput")
        elif mode == "f16d":
            nv = 3 if name.startswith("W20") else 4
            for j in range(nv):
                dram_w[f"{name}d{j}"] = nc.dram_tensor(
                    f"{name}d{j}", shape, f16, kind="ExternalInput")
        else:
            dram_w[name] = nc.dram_tensor(name, shape, wdt(mode),
                                          kind="ExternalInput")

    wdecl("w0", [DRNN, DHID], M_Z1)
    wdecl("w1a", [128, DHID], M_Z2)
    wdecl("w1b", [128, DHID], M_Z2)
    wdecl("w2a", [128, DRNN], M_W20)
    wdecl("w2b", [128, DRNN], M_W20)
    wdecl("W20a", [128, DHID], M_W20)
    wdecl("W20b", [128, DHID], M_W20)
    wdecl("whh0", [DRNN, 3 * DRNN], M_G0H)
    wdecl("whh1", [DRNN, 3 * DRNN], M_G1H)   # z-gate cols negated
    wdecl("wih1", [DRNN, 3 * DRNN], M_G1I)   # z-gate cols negated
    FUSE_NEG = os.environ.get("ODERNN_FUSENEG", "1") == "1" \
        and M_G1I == "f32" and M_Z1 == "f32"
    if FUSE_NEG:
        wdecl("wih1n", [DRNN, 3 * DRNN], M_G1I)  # -wih1n for -p0 moving
        wdecl("w0n", [DRNN, DHID], M_Z1)         # -w0 for -q moving
    dram_w["wih6"] = nc.dram_tensor("wih6", [6, 3 * DRNN], f16,
                                    kind="ExternalInput")
    dram_w["wouth"] = nc.dram_tensor("wouth", [DRNN, DOUT], f16,
                                     kind="ExternalInput")
    d_bias = nc.dram_tensor("biaspk", [DRNN, 24], f32, kind="ExternalInput")
    d_y = nc.dram_tensor("y", [DOUT, SR], f32, kind="ExternalOutput")

    with tile.TileContext(nc) as tc:
        wp = tc.alloc_tile_pool(name="wconst", bufs=1)
        st = tc.alloc_tile_pool(name="state", bufs=4)
        wk = tc.alloc_tile_pool(name="work", bufs=6)
        pp = tc.alloc_tile_pool(name="psum", bufs=1, space="PSUM")

        def load(dram, shape, dtype, name):
            t = wp.tile(shape, dtype, tag=name, name=name)
            nc.sync.dma_start(out=t[:], in_=dram[:])
            return t

        W = {}
        for nm, dten in dram_w.items():
            W[nm] = load(dten, list(dten.shape), dten.dtype, nm)
        xm6 = load(d_xm6, [6, SR], f16, "xm6")
        h0f = load(d_h0f, [DRNN, R], f32, "h0f")
        h0h = load(d_h0h, [DRNN, R], f16, "h0h")
        bias = load(d_bias, [DRNN, 24], f32, "biaspk")
        need_hl = M_Z1 == "f16x3"
        need_h1l = "f16x3" in (M_G0H, M_G1H)
        h0l = load(d_h0l, [DRNN, R], f16, "h0l") if need_hl else None
        mm16 = load(d_mm, [DRNN, SR], f16, "mm16")

        hist = wp.tile([DRNN, NSTEPS, R], f16, tag="hist", name="hist")

        MM = nc.tensor.matmul
        ACT = nc.scalar.activation
        V = nc.vector
        GP = nc.gpsimd if os.environ.get("ODERNN_GP", "1") == "1" else nc.vector
        _gpops = set(os.environ.get("ODERNN_GPOPS", "ysl").split(","))

        def ENG(name):
            return nc.gpsimd if name in _gpops else nc.vector

        # ---- PSUM: NSLOT slots x {tz1 (2 banks), tAB (2 banks)} ----
        # Long-open groups (z1 blocks, hd) own their banks; quick pairs
        # share strictly back-to-back.  prz1 blocks live in SEPARATE banks
        # (same region index of tAB's two banks) so both can be opened
        # early by the whh1 matmuls while staying one-open-per-bank, and
        # the merged rz1 ACT still sees one strided AP.
        NSLOT = 2
        slots = []
        for si in range(NSLOT):
            tz1 = pp.tile([128, 2, 512], f32, tag=f"tz1_{si}",
                          name=f"tz1_{si}")
            tAB = pp.tile([128, 2, 4, 128], f32, tag=f"tAB_{si}",
                          name=f"tAB_{si}")
            slots.append({
                "z1": tz1[:, :, 0:RG],
                "z1b": [tz1[:, 0, 0:RG], tz1[:, 1, 0:RG]],
                "prz0": tz1[:, :, 128:128 + RG],
                "prz0b": [tz1[:, 0, 128:128 + RG], tz1[:, 1, 128:128 + RG]],
                "pg0_ig": tz1[:, 0, 256:256 + RG],
                "pg0_hg": tz1[:, 1, 256:256 + RG],
                "pg1_ig": tz1[:, 0, 384:384 + RG],
                "z2": tAB[:, 0, 0:2, 0:RG],
                "z2b": [tAB[:, 0, 0, 0:RG], tAB[:, 0, 1, 0:RG]],
                "prz1": tAB[:, :, 2, 0:RG],
                "prz1b": [tAB[:, 0, 2, 0:RG], tAB[:, 1, 2, 0:RG]],
                "hd": tAB[:, 1, 0, 0:RG],
                "pg1_hg": tAB[:, 1, 1, 0:RG],
                "py": tAB[0:DOUT, 1, 3, 0:RG],
            })

        def site_mm(out, wname, blk, mov_f32, mov_h, mov_l, mode,
                    start, stop, par=0):
            lo, hi = blk
            if mode == "f32":
                MM(out, W[wname][:, lo:hi], mov_f32, start=start, stop=stop)
            elif mode == "f16":
                MM(out, W[wname][:, lo:hi], mov_h, start=start, stop=stop)
            elif mode == "f16d":
                nv = 3 if wname.startswith("W20") else 4
                MM(out, W[f"{wname}d{par % nv}"][:, lo:hi],
                   mov_h, start=start, stop=stop)
            elif mode == "f16x2w":
                MM(out, W[wname + "h"][:, lo:hi], mov_h, start=start,
                   stop=False)
                MM(out, W[wname + "l"][:, lo:hi], mov_h, start=False,
                   stop=stop)
            else:  # f16x3
                MM(out, W[wname + "h"][:, lo:hi], mov_h, start=start,
                   stop=False)
                MM(out, W[wname + "h"][:, lo:hi], mov_l, start=False,
                   stop=False)
                MM(out, W[wname + "l"][:, lo:hi], mov_h, start=False,
                   stop=stop)

        a1dt = f32 if M_Z2 == "f32" else f16
        a2dt = f32 if M_W20 == "f32" else f16

        def group_body(gi):
            """Generator emitting one group's full sequence; yields at
            phase boundaries for cross-group interleaving."""
            hpf = h0f[:, gi * RG:(gi + 1) * RG]
            hph = h0h[:, gi * RG:(gi + 1) * RG]
            hpl = h0l[:, gi * RG:(gi + 1) * RG] if need_hl else None

            for s in range(NSTEPS):
                t = slots[(s * G + gi) % NSLOT]
                c0 = s * R + gi * RG
                c1 = c0 + RG
                xsl = xm6[:, c0:c1]
                msl = mm16[:, c0:c1]

                # ph0: open z1 telescopes.  For s>0 with M_Z1=f32 this
                # already happened during the previous step's tail:
                # z1 = w0^T h_next = w0^T q + w0^T p1 distributed through
                # the PE, so the chain never waits on an h_next vector op.
                if s == 0:
                    site_mm(t["z1b"][0], "w0", (0, 128), hpf, hph, hpl,
                            M_Z1, True, False)
                    site_mm(t["z1b"][1], "w0", (128, 256), hpf, hph, hpl,
                            M_Z1, True, False)
                yield

                for k in range(K):
                    last = k == K - 1
                    a1 = wk.tile([128, 2, RG], a1dt, tag=f"a1_{gi}")
                    if not fb01:
                        ACT(a1[:], t["z1"], Act.Tanh)
                    else:
                        ACT(a1[:, 0, :], t["z1b"][0], Act.Tanh,
                            bias=bias[:, 16 + 2 * k:17 + 2 * k])
                        ACT(a1[:, 1, :], t["z1b"][1], Act.Tanh,
                            bias=bias[:, 17 + 2 * k:18 + 2 * k])
                    yield
                    a1h = [a1[:, 0, :], a1[:, 1, :]]
                    par = (s * K + k) & 3
                    site_mm(t["z2b"][0], "w1a", (0, 128), a1h[0], a1h[0],
                            None, M_Z2, True, False, par)
                    site_mm(t["z2b"][0], "w1b", (0, 128), a1h[1], a1h[1],
                            None, M_Z2, False, True, par)
                    site_mm(t["z2b"][1], "w1a", (128, 256), a1h[0], a1h[0],
                            None, M_Z2, True, False, par)
                    site_mm(t["z2b"][1], "w1b", (128, 256), a1h[1], a1h[1],
                            None, M_Z2, False, True, par)
                    yield
                    a2 = wk.tile([128, 2, RG], a2dt, tag=f"a2_{gi}")
                    if not fb01:
                        ACT(a2[:], t["z2"], Act.Tanh)
                    else:
                        ACT(a2[:, 0, :], t["z2b"][0], Act.Tanh,
                            bias=bias[:, 2:3])
                        ACT(a2[:, 1, :], t["z2b"][1], Act.Tanh,
                            bias=bias[:, 3:4])
                    yield
                    a2h = [a2[:, 0, :], a2[:, 1, :]]
                    if not last:
                        fin = k == K - 2
                        site_mm(t["z1b"][0], "W20a", (0, 128), a2h[0],
                                a2h[0], None, M_W20, False, False, par)
                        site_mm(t["z1b"][0], "W20b", (0, 128), a2h[1],
                                a2h[1], None, M_W20, False, fin, par)
                        site_mm(t["z1b"][1], "W20a", (128, 256), a2h[0],
                                a2h[0], None, M_W20, False, False, par)
                        site_mm(t["z1b"][1], "W20b", (128, 256), a2h[1],
                                a2h[1], None, M_W20, False, fin, par)
                    site_mm(t["hd"], "w2a", (0, 128), a2h[0], a2h[0], None,
                            M_W20, k == 0, False, par)
                    site_mm(t["hd"], "w2b", (0, 128), a2h[1], a2h[1], None,
                            M_W20, False, last, par)
                    yield

                # h1 = h_prev + (hd + DT*b2); the copy consumed by the
                # whh0 chain matmuls is emitted FIRST on DVE
                h1h = hist[:, s, gi * RG:(gi + 1) * RG]
                h1f = st.tile([128, RG], f32, tag=f"h1f_{gi}")
                stts = [h1h, h1f[:]]
                if M_G0H == "f32":
                    stts.reverse()
                for dst in stts:
                    V.scalar_tensor_tensor(dst, t["hd"], bias[:, 4:5], hpf,
                                           op0=Alu.add, op1=Alu.add)
                h1l = None
                if need_h1l:
                    h1lt = wk.tile([128, RG], f16, tag=f"h1l_{gi}")
                    GP.tensor_sub(h1lt[:], h1f[:], h1h)
                    h1l = h1lt[:]
                yield

                # GRU0 matmuls + GRU1 g-gate hidden side + wout projection
                MM(t["pg0_ig"], W["wih6"][:, 256:384], xsl, start=True,
                   stop=True)
                # dep-free x-openers first (the two prz0 blocks live in
                # different banks, so both groups may be open at once);
                # the h1f-gated whh0 closers run back-to-back after
                MM(t["prz0b"][0], W["wih6"][:, 0:128], xsl, start=True,
                   stop=False)
                MM(t["prz0b"][1], W["wih6"][:, 128:256], xsl, start=True,
                   stop=False)
                site_mm(t["prz0b"][0], "whh0", (0, 128), h1f[:], h1h, h1l,
                        M_G0H, False, True)
                site_mm(t["prz0b"][1], "whh0", (128, 256), h1f[:], h1h, h1l,
                        M_G0H, False, True)
                site_mm(t["pg0_hg"], "whh0", (256, 384), h1f[:], h1h, h1l,
                        M_G0H, True, True)
                site_mm(t["pg1_hg"], "whh1", (256, 384), h1f[:], h1h, h1l,
                        M_G1H, True, True)
                MM(t["py"], W["wouth"][:], h1h, start=True, stop=True)
                # open GRU1 r/z groups on the h1-dependent halves now; the
                # wih1 input-side terms arrive as v0/p0 products later
                # (h2 = u0*n0 + zz0*h1 distributed through the PE)
                site_mm(t["prz1b"][0], "whh1", (0, 128), h1f[:], h1h, h1l,
                        M_G1H, True, False)
                site_mm(t["prz1b"][1], "whh1", (128, 256), h1f[:], h1h, h1l,
                        M_G1H, True, False)
                yield

                rz0 = wk.tile([128, 2, RG], f32, tag=f"rz0_{gi}")
                if not frz0:
                    ACT(rz0[:], t["prz0"], Act.Sigmoid)
                else:
                    ACT(rz0[:, 0, :], t["prz0b"][0], Act.Sigmoid,
                        bias=bias[:, 5:6])
                    ACT(rz0[:, 1, :], t["prz0b"][1], Act.Sigmoid,
                        bias=bias[:, 6:7])
                r0, zz0 = rz0[:, 0, :], rz0[:, 1, :]
                yield

                hg0 = t["pg0_hg"]
                if fg0h:
                    hg0t = wk.tile([128, RG], f32, tag=f"hg0t_{gi}")
                    nc.scalar.add(hg0t[:], hg0, bias[:, 8:9])
                    hg0 = hg0t[:]
                # chain: s1 -> np0 -> [n0]; u0/v precomputed for the tail
                s1 = wk.tile([128, RG], f32, tag=f"s1_{gi}")
                V.tensor_mul(s1[:], r0, hg0)
                np0 = wk.tile([128, RG], f32, tag=f"np0_{gi}")
                V.tensor_add(np0[:], s1[:], t["pg0_ig"])
                if not FUSE_NEG:
                    u0 = wk.tile([128, RG], f32, tag=f"u0_{gi}")
                    ENG("u0").tensor_scalar(u0[:], zz0, -1.0, 1.0,
                                            op0=Alu.mult, op1=Alu.add)
                g1dt = f32 if M_G1I == "f32" else f16
                v0 = wk.tile([128, RG], g1dt, tag=f"v0_{gi}")
                ENG("v0").tensor_mul(v0[:], zz0, h1f[:])
                yield

                n0 = wk.tile([128, RG], f32, tag=f"n0_{gi}")
                if not fg0i:
                    ACT(n0[:], np0[:], Act.Tanh)
                else:
                    ACT(n0[:], np0[:], Act.Tanh, bias=bias[:, 7:8])
                # v0-half of the GRU1 input-side products
                site_mm(t["prz1b"][0], "wih1", (0, 128), v0[:], v0[:],
                        None, M_G1I, False, False)
                site_mm(t["prz1b"][1], "wih1", (128, 256), v0[:], v0[:],
                        None, M_G1I, False, False)
                site_mm(t["pg1_ig"], "wih1", (256, 384), v0[:], v0[:],
                        None, M_G1I, True, False)
                yield

                # only p0 sits on the chain after n0; fused form computes
                # p0n = (zz0 - 1)*n0 = -p0 in one op (wih1n mms negate back)
                p0 = wk.tile([128, RG], g1dt, tag=f"p0_{gi}")
                if FUSE_NEG:
                    ENG("p0").scalar_tensor_tensor(p0[:], zz0, 1.0, n0[:],
                                                   op0=Alu.subtract,
                                                   op1=Alu.mult)
                else:
                    ENG("p0").tensor_mul(p0[:], u0[:], n0[:])
                # y slice out (off critical path, backfills this phase)
                ysl = wk.tile([DOUT, RG], f32, tag=f"ysl_{gi}")
                if not fbout:
                    if "ysl" in _gpops:
                        ACT(ysl[:], t["py"], Act.Identity)
                    else:
                        V.tensor_copy(ysl[:], t["py"])
                else:
                    ACT(ysl[:], t["py"], Act.Identity,
                        bias=bias[0:DOUT, 15:16])
                nc.sync.dma_start(out=d_y[:, c0:c1], in_=ysl[:])
                yield

                # close GRU1 groups with the p0-half products
                _wi = "wih1n" if FUSE_NEG else "wih1"
                site_mm(t["prz1b"][0], _wi, (0, 128), p0[:], p0[:],
                        None, M_G1I, False, True)
                site_mm(t["prz1b"][1], _wi, (128, 256), p0[:], p0[:],
                        None, M_G1I, False, True)
                site_mm(t["pg1_ig"], _wi, (256, 384), p0[:], p0[:],
                        None, M_G1I, False, True)
                yield

                rz1 = wk.tile([128, 2, RG], f32, tag=f"rz1_{gi}")
                if not frz1:
                    ACT(rz1[:], t["prz1"], Act.Sigmoid)
                else:
                    ACT(rz1[:, 0, :], t["prz1b"][0], Act.Sigmoid,
                        bias=bias[:, 9:10])
                    ACT(rz1[:, 1, :], t["prz1b"][1], Act.Sigmoid,
                        bias=bias[:, 10:11])
                r1, u1 = rz1[:, 0, :], rz1[:, 1, :]
                yield

                hg1 = t["pg1_hg"]
                if fg1h:
                    hg1t = wk.tile([128, RG], f32, tag=f"hg1t_{gi}")
                    nc.scalar.add(hg1t[:], hg1, bias[:, 12:13])
                    hg1 = hg1t[:]
                # chain: s2 -> np1 -> [n1]; gm/gh/q precomputed for the tail
                s2 = wk.tile([128, RG], f32, tag=f"s2_{gi}")
                V.tensor_mul(s2[:], r1, hg1)
                np1 = wk.tile([128, RG], f32, tag=f"np1_{gi}")
                V.tensor_add(np1[:], s2[:], t["pg1_ig"])
                gm = wk.tile([128, RG], f32, tag=f"gm_{gi}")
                ENG("gm").tensor_mul(gm[:], u1, msl)
                q = st.tile([128, RG], f32, tag=f"q_{gi}")
                if FUSE_NEG:
                    # q holds -q = (gm - 1)*h1f; w0n mms negate back
                    ENG("q").scalar_tensor_tensor(q[:], gm[:], 1.0, h1f[:],
                                                  op0=Alu.subtract,
                                                  op1=Alu.mult)
                else:
                    gh = wk.tile([128, RG], f32, tag=f"gh_{gi}")
                    ENG("gh").tensor_mul(gh[:], gm[:], h1f[:])
                    ENG("q").tensor_sub(q[:], h1f[:], gh[:])
                if M_Z1 == "f32":
                    qm = q
                else:
                    qm = st.tile([128, RG], f16, tag=f"q16_{gi}")
                    V.tensor_copy(qm[:], q[:])
                if s + 1 < NSTEPS:
                    # open next step's z1 telescope early on the q-half
                    _w0 = "w0n" if FUSE_NEG else "w0"
                    tn = slots[((s + 1) * G + gi) % NSLOT]
                    site_mm(tn["z1b"][0], _w0, (0, 128), qm[:], qm[:], None,
                            M_Z1, True, False)
                    site_mm(tn["z1b"][1], _w0, (128, 256), qm[:], qm[:],
                            None, M_Z1, True, False)
                yield

                n1 = wk.tile([128, RG], f32, tag=f"n1_{gi}")
                if not fg1i:
                    ACT(n1[:], np1[:], Act.Tanh)
                else:
                    ACT(n1[:], np1[:], Act.Tanh, bias=bias[:, 11:12])
                yield

                # h_next = h1*(1-g) + g*n1 = q + g*n1, g = mm*u1
                z1dt = f32 if M_Z1 == "f32" else f16
                p1 = st.tile([128, RG], z1dt, tag=f"p1_{gi}")
                ENG("p1").tensor_mul(p1[:], gm[:], n1[:])
                if s + 1 < NSTEPS:
                    # p1-half of next step's z1 telescope (chain never
                    # touches hn itself)
                    tn = slots[((s + 1) * G + gi) % NSLOT]
                    site_mm(tn["z1b"][0], "w0", (0, 128), p1[:], p1[:],
                            None, M_Z1, False, False)
                    site_mm(tn["z1b"][1], "w0", (128, 256), p1[:], p1[:],
                            None, M_Z1, False, False)
                if M_Z1 == "f32":
                    p1x = p1
                else:
                    p1x = st.tile([128, RG], f32, tag=f"p1x_{gi}")
                    GP.tensor_mul(p1x[:], gm[:], n1[:])
                hn = st.tile([128, RG], f32, tag=f"hn_{gi}")
                if FUSE_NEG:
                    ENG("hn").tensor_sub(hn[:], p1x[:], q[:])
                else:
                    ENG("hn").tensor_add(hn[:], q[:], p1x[:])
                hpf = hn[:]
                yield

        # staggered round-robin phase interleave across groups
        PH = 11 + 4 * K         # yields per step
        STAG = int(os.environ.get("ODERNN_STAG", "20"))
        gens = [group_body(gi) for gi in range(G)]
        started = [False] * G
        done = [False] * G
        tick = 0
        while not all(done):
            for gi in range(G):
                if tick >= gi * STAG and not done[gi]:
                    started[gi] = True
                    try:
                        next(gens[gi])
                    except StopIteration:
                        done[gi] = True
            tick += 1

        pp.release()
        wk.release()
        st.release()
        wp.release()

    nc.compile()
    return nc


def _prep(inputs):
    x2d = np.asarray(inputs["x2d"], np.float32)
    mask = np.asarray(inputs["mask"])
    g = lambda n: np.asarray(inputs[n], np.float32)
    w0, b0 = g("ode_w0"), g("ode_b0")
    w1, b1 = g("ode_w1"), g("ode_b1")
    w2, b2 = g("ode_w2"), g("ode_b2")
    wih0, whh0 = g("wih0"), g("whh0")
    bih0, bhh0 = g("bih0"), g("bhh0")
    wih1, whh1 = g("wih1"), g("whh1")
    bih1, bhh1 = g("bih1"), g("bhh1")
    wout, bout = g("wout"), g("bout")
    h0 = g("h0")

    mf = mask.astype(np.float32)
    xs = (x2d * mf).reshape(B, S, N, DIN)[:, :NSTEPS]
    ms = mf.reshape(B, S, N)[:, :NSTEPS]

    W20 = (DT * (w2.astype(np.float64) @ w0.astype(np.float64))).astype(np.float32)
    h0T = np.repeat(h0.reshape(DRNN, 1), R, axis=1).astype(np.float32)
    h0h, h0l = _split16(h0T)

    whh1n = whh1.copy(); whh1n[:, 128:256] *= -1.0
    wih1n = wih1.copy(); wih1n[:, 128:256] *= -1.0

    bp = np.zeros((DRNN, 24), np.float32)
    bp[:, 0], bp[:, 1] = b0[0:128], b0[128:256]
    bp[:, 2], bp[:, 3] = b1[0:128], b1[128:256]
    bp[:, 4] = DT * b2
    brz0 = bih0 + bhh0
    bp[:, 5], bp[:, 6] = brz0[0:128], brz0[128:256]
    bp[:, 7] = bih0[256:384]
    bp[:, 8] = bhh0[256:384]
    brz1 = bih1 + bhh1
    bp[:, 9] = brz1[0:128]
    bp[:, 10] = -brz1[128:256]          # z-gate negated
    bp[:, 11] = bih1[256:384]
    bp[:, 12] = bhh1[256:384]
    bp[0:DOUT, 15] = bout
    zb = DT * (b2 @ w0)
    for k in range(K):
        bp[:, 16 + 2 * k + 0] = b0[0:128] + k * zb[0:128]
        bp[:, 16 + 2 * k + 1] = b0[128:256] + k * zb[128:256]

    flags = (
        bool(np.any(b0) or np.any(b1) or np.any(b2)),
        bool(np.any(b2)),
        bool(np.any(brz0[0:256])),
        bool(np.any(bhh0[256:384])),
        bool(np.any(bih0[256:384])),
        bool(np.any(brz1[0:256])),
        bool(np.any(bhh1[256:384])),
        bool(np.any(bih1[256:384])),
        bool(np.any(bout)),
    )

    C = np.ascontiguousarray

    def wpack(name, arr, mode):
        out = {}
        if mode in ("f16x2w", "f16x3"):
            hi, lo = _split16(arr)
            out[name + "h"] = C(hi)
            out[name + "l"] = C(lo)
        elif mode == "f16":
            out[name] = C(arr.astype(F16))
        elif mode == "f16d":
            nv = 3 if name.startswith("W20") else 4
            for j, v in enumerate(_dither16(arr, nv)):
                out[f"{name}d{j}"] = C(v)
        else:
            out[name] = C(arr.astype(np.float32))
        return out

    shared = {}
    shared.update(wpack("w0", w0, M_Z1))
    shared.update(wpack("w1a", w1[0:128], M_Z2))
    shared.update(wpack("w1b", w1[128:256], M_Z2))
    shared.update(wpack("w2a", DT * w2[0:128], M_W20))
    shared.update(wpack("w2b", DT * w2[128:256], M_W20))
    shared.update(wpack("W20a", W20[0:128], M_W20))
    shared.update(wpack("W20b", W20[128:256], M_W20))
    shared.update(wpack("whh0", whh0, M_G0H))
    shared.update(wpack("whh1", whh1n, M_G1H))
    shared.update(wpack("wih1", wih1n, M_G1I))
    if os.environ.get("ODERNN_FUSENEG", "1") == "1" \
            and M_G1I == "f32" and M_Z1 == "f32":
        shared.update(wpack("wih1n", -wih1n, M_G1I))
        shared.update(wpack("w0n", -w0, M_Z1))
    wih0h, wih0l = _split16(wih0)
    shared["wih6"] = C(np.concatenate([wih0h, wih0h, wih0l], axis=0))
    shared["wouth"] = C(wout.astype(F16))
    shared["biaspk"] = bp
    shared["h0f"] = h0T
    shared["h0h"] = C(h0h)
    shared["h0l"] = C(h0l)

    in_maps = []
    for c in range(NCORES):
        xc = xs[c * BL:(c + 1) * BL]
        xmT = xc.transpose(3, 1, 0, 2).reshape(DIN, SR)
        xh, xl = _split16(xmT)
        xm6 = np.concatenate([xh, xl, xh], axis=0)
        mc = ms[c * BL:(c + 1) * BL]
        mrow = mc.transpose(1, 0, 2).reshape(1, SR)
        mbc = np.broadcast_to(mrow, (DRNN, SR)).astype(F16)
        m = dict(shared)
        m["xm6"] = C(xm6)
        m["mm16"] = C(mbc)
        in_maps.append(m)
    return in_maps, flags


def kernel(**inputs):
    in_maps, flags = _prep(inputs)
    if flags not in _prog_cache:
        _prog_cache[flags] = _build_program(flags)
    nc = _prog_cache[flags]

    from concourse.bass_utils import run_bass_kernel_spmd
    res = run_bass_kernel_spmd(nc, in_maps, core_ids=list(range(NCORES)))
    global _last_results
    _last_results = res.results

    ys = np.zeros((B, NSTEPS, P, J, DOUT), np.float32)
    for c in range(NCORES):
        y = res.results[c]["y"]                      # (DOUT, SR)
        y = y.reshape(DOUT, NSTEPS, BL, N).transpose(2, 1, 3, 0)
        ys[c * BL:(c + 1) * BL] = y.reshape(BL, NSTEPS, P, J, DOUT)
    return ys

